# revision 1
# baseline (speedup 1.0000x reference)
"""CRF loss (nn_CRFLossOld) on 8 Trainium2 NeuronCores.

Data-parallel over the batch axis: each of the 8 cores processes 128
sequences. The sequential CRF forward recurrence is computed in the
LINEAR domain as a chain of 64x65 PE matmuls with the exp'd transition
matrix as the stationary operand:

    a_j = (E_aug^T a_{j-1})[0:64] * exp(obs_j)        (E_aug = [E | 1])

The 65th output row of each matmul is the column-sum of a_{j-1}, used
for periodic per-sequence rescaling (applied with a small lag to a
future exp(obs) tile, off the critical path). Exact bookkeeping:
logZ_b = ln(zacc_b) + shift_b - 1000, where zacc accumulates row 63
(the end label) -- nonzero exactly once per sequence, at step
seq_len+1 -- and is multiplied by every subsequent rescale factor,
while shift accumulates ln(colsum) of the same rescales; the product
telescopes exactly.

Host-side prep is integer-derived only (dtype-preserving copies,
constant fills, one-hot masks, pair counts); all float arithmetic on
pred/transitions happens on-device.

Key numerical facts mirrored from the reference (validated vs fp64):
  - init alpha = b_s has -1000 padding mass at all emission labels,
    which dominates the actual start-label mass (whose outgoing
    transitions are -10000): init a_0 = ones on labels 0..61.
  - transitions rows 62/63 are exactly -10000; on device they get a
    +10000 bias before exp (E rows 62/63 become 1.0). These rows only
    ever touch dead/finished columns (keep-alive plumbing) and never
    affect live probability mass.
"""

import os
import sys

for _p in ("/opt/trn_rl_repo", "/root/.axon_site/_ro/trn_rl_repo"):
    if os.path.isdir(_p) and _p not in sys.path:
        sys.path.insert(0, _p)

import numpy as np

B, T, L = 1024, 512, 62
K = 64
NCORES = 8
BC = B // NCORES            # 128 sequences per core
NJ = T + 1                  # 513 recurrence steps (j = 1..513)
NJP = (NJ + 1) // 2         # 257 packed step-pairs (partition parity)
JP_CHUNK = 32               # step-pairs per DMA/exp chunk (64 steps)
GOLD_SUB = 8                # step-pairs per gold-reduce piece
R_RESCALE = 8               # rescale decision every R steps
LAG = 3                     # applied to the eobs tile LAG steps later


def _rescale_schedule():
    """(decision_step, apply_step) pairs, trace-time constant."""
    out = []
    for jd in range(R_RESCALE, NJ + 1, R_RESCALE):
        if jd + LAG <= NJ:
            out.append((jd, jd + LAG))
    return out

_PROGRAM_CACHE = {}


# --------------------------------------------------------------------------
# host-side packing (integer-derived only)
# --------------------------------------------------------------------------

def _build_host_tensors(pred, ref, seq_len, fp8_dtype):
    """Packed obs (f32), one-hot (fp8) and transition-pair counts (f32).

    Packed layout per core: [jp, 64*parity + k', b] with step index
    j = 2*jp + parity + 1 (jj = j-1 = 2*jp + parity), so each SBUF tile
    holds two recurrence steps across its 128 partitions.

    Labels are permuted so partition slices start at 0/64 (HW requires
    start partition in {0,32,64,96}): k'=0 end label, k'=1 start label,
    k'=2+l emission label l. Returns (obsP, ohP, cmat', trans_perm).
    """
    pred = np.ascontiguousarray(pred, dtype=np.float32)
    ref = np.asarray(ref).astype(np.int64)
    seq_len = np.asarray(seq_len).astype(np.int64)

    pred_r = pred.reshape(NCORES, BC, T, L)
    seq_r = seq_len.reshape(NCORES, BC)
    ref_r = ref.reshape(NCORES, BC, T)

    obsP = np.full((NCORES, NJP, 128, BC), -1000.0, dtype=np.float32)
    for parity in (0, 1):
        jpn = (NJ - parity + 1) // 2          # jp slots for this parity
        jj = parity + 2 * np.arange(jpn)      # jj = j-1 in 0..512
        # live emission rows: j <= seq  <=>  jj < seq ; t = jj
        t_rows = jj[jj < T]                   # emission rows exist for t < T
        jp_rows = (t_rows - parity) // 2
        live = t_rows[None, :, None] < seq_r[:, None, :]        # (C, n, BC)
        vals = pred_r[:, :, t_rows, :].transpose(0, 2, 3, 1)    # (C, n, L, BC)
        obsP[:, jp_rows, 64 * parity + 2 : 64 * parity + 2 + L, :] = np.where(
            live[:, :, None, :], vals, np.float32(-1000.0)
        )
        # death rows (j > seq+1  <=>  jj > seq): k'=1 -> 0.0 (keep-alive)
        dead = jj[None, :, None] > seq_r[:, None, :]            # (C, jpn, BC)
        obsP[:, (jj - parity) // 2, 64 * parity + 1, :] = np.where(
            dead, np.float32(0.0), np.float32(-1000.0)
        )
    # extraction row: jj == seq -> k'=0 (end label) = 0.0
    c_idx = np.repeat(np.arange(NCORES), BC)
    b_idx = np.tile(np.arange(BC), NCORES)
    s_flat = seq_r.reshape(-1)
    obsP[c_idx, s_flat // 2, 64 * (s_flat % 2) + 0, b_idx] = 0.0

    # one-hot gold emissions, same packing (jj = t), zero where t >= seq
    ohP = np.zeros((NCORES, NJP, 128, BC), dtype=fp8_dtype)
    tt = np.arange(T)
    cg, bg, tg = np.nonzero(tt[None, None, :] < seq_r[:, :, None])
    labels = ref_r[cg, bg, tg]
    ohP[cg, tg // 2, 64 * (tg % 2) + 2 + labels, bg] = fp8_dtype(1.0)

    # transition-pair counts per core in permuted space:
    # path' = [start'=1, ref+2 ..., end'=0]
    cmat = np.zeros((NCORES, K, K), dtype=np.int64)
    for c in range(NCORES):
        for b in range(BC):
            s = int(seq_r[c, b])
            path = np.concatenate(([1], ref_r[c, b, :s] + 2, [0]))
            np.add.at(cmat[c], (path[:-1], path[1:]), 1)

    # shift snapshot mask: M[r, b] = 1 iff rescale r was applied at or
    # before this sequence's extraction step (apply_step <= seq_len+1),
    # i.e. its ln(colsum) is part of the deposit's accumulated scale.
    sched = _rescale_schedule()
    ja = np.array([a for (_, a) in sched])                      # (NR,)
    smask = (ja[None, None, :] <= (seq_r[:, :, None] + 1)).astype(np.float32)
    smask = np.ascontiguousarray(smask.transpose(0, 2, 1))     # (C, NR, BC)
    return obsP, ohP, cmat.astype(np.float32), smask


# permutation: new index k' -> old label index
PERM = np.concatenate(([63, 62], np.arange(62)))


# --------------------------------------------------------------------------
# device program
# --------------------------------------------------------------------------

def _build_program(reps=1):
    import concourse.bacc as bacc
    import concourse.tile as tile
    from concourse import mybir

    f32 = mybir.dt.float32
    u8 = mybir.dt.uint8
    AF = mybir.ActivationFunctionType
    ALU = mybir.AluOpType
    AX = mybir.AxisListType

    nc = bacc.Bacc()
    obs_d = nc.dram_tensor("obs", [NJP, 128, BC], f32, kind="ExternalInput")
    oh_d = nc.dram_tensor("oh", [NJP, 128, BC], u8, kind="ExternalInput")
    trans_d = nc.dram_tensor("trans", [K, K], f32, kind="ExternalInput")
    cmat_d = nc.dram_tensor("cmat", [K, K], f32, kind="ExternalInput")
    nr = len(_rescale_schedule())
    out_d = nc.dram_tensor("out", [1, 4], f32, kind="ExternalOutput")

    nchunk = (NJP + JP_CHUNK - 1) // JP_CHUNK

    with tile.TileContext(nc) as tc:
        with (
            tc.tile_pool(name="const", bufs=1) as const,
            tc.tile_pool(name="obsch", bufs=3) as obsch,
            tc.tile_pool(name="ohch", bufs=2) as ohch,
            tc.tile_pool(name="eobsch", bufs=4) as eobsch,
            tc.tile_pool(name="apool", bufs=3) as apool,
            tc.tile_pool(name="spool", bufs=2) as spool,
            tc.tile_pool(name="rpool", bufs=2) as rpool,
            tc.tile_pool(name="gpool", bufs=2) as gpool,
            tc.tile_pool(name="gprod", bufs=2) as gprod,
            tc.tile_pool(name="endp", bufs=1) as endp,
            tc.tile_pool(name="pchain", bufs=2, space="PSUM") as pchain,
            tc.tile_pool(name="pmisc", bufs=2, space="PSUM") as pmisc,
        ):
            # ---- constants -------------------------------------------------
            # reps>1 repeats the whole computation for in-NEFF timing
            trans_s = const.tile([K, K], f32)
            nc.gpsimd.dma_start(out=trans_s, in_=trans_d[:, :])
            cmat_s = const.tile([K, K], f32)
            nc.gpsimd.dma_start(out=cmat_s, in_=cmat_d[:, :])

            # E = exp(trans_perm); rows 0/1 (from-end / from-start, exactly
            # -10000 in the input) are overwritten with the +10000-bias
            # equivalent exp(0)=1 -- pure keep-alive plumbing, never touches
            # live mass. 65th column = ones (free column-sum of a).
            e_aug = const.tile([K, K + 1], f32)
            nc.scalar.activation(
                out=e_aug[:, 0:K], in_=trans_s, func=AF.Exp,
            )
            nc.vector.memset(e_aug[0:2, 0:K], 1.0)
            nc.vector.memset(e_aug[:, K : K + 1], 1.0)

            # gold transition score sum_ij cmat*trans -> (K,1), done early
            # while engines are idle (few outstanding semaphores).
            # stage both through ScalarE copies: TensorTensor ISA slots can't
            # encode DMA-semaphore waits, so give the mul a single engine-sem
            # dependency instead.
            trans_st = const.tile([K, K], f32)
            nc.scalar.copy(out=trans_st, in_=trans_s)
            cmat_st = const.tile([K, K], f32)
            nc.scalar.copy(out=cmat_st, in_=cmat_s)
            scr = const.tile([K, K], f32)
            nc.vector.tensor_mul(scr, trans_st, cmat_st)
            gt = const.tile([K, 1], f32)
            nc.vector.tensor_reduce(out=gt, in_=scr, axis=AX.X, op=ALU.add)

            ones_w = const.tile([128, K], f32)      # row 64 = bcast weights
            nc.vector.memset(ones_w, 1.0)
            ones_col = const.tile([128, 1], f32)
            nc.vector.memset(ones_col, 1.0)
            # two-hot column [1,1,0,...]: final w_b = a_NJ[0,b] + a_NJ[1,b]
            # (exactly one of the two is nonzero per sequence)
            e01col = const.tile([K, 1], f32)
            nc.vector.memset(e01col, 0.0)
            nc.vector.memset(e01col[0:2, :], 1.0)
            lnbuf = const.tile([1, nr, BC], f32)

            a0A = const.tile([K, BC // 2], f32)
            nc.vector.memset(a0A, 1.0)
            nc.vector.memset(a0A[0:2, :], 0.0)
            a0B = const.tile([K, BC // 2], f32)
            nc.vector.memset(a0B, 1.0)
            nc.vector.memset(a0B[0:2, :], 0.0)

            # ---- body (repeated `reps` times for timing builds) -----------
            for _rep in range(reps):
              gacc_t = const.tile([128, 1], f32, tag="gacc_t")
              nc.vector.memset(gacc_t, 0.0)
                # ---- streamed chunks: DMA -> exp (ACT) + gold (PL/ACT)
              eobs_tiles = []
              for c in range(nchunk):
                  jp0 = c * JP_CHUNK
                  cw = min(JP_CHUNK, NJP - jp0)
                  ob = obsch.tile([128, JP_CHUNK, BC], f32, tag="ob")
                  nc.sync.dma_start(
                      out=ob[:, :cw, :],
                      in_=obs_d[jp0 : jp0 + cw].rearrange("j p b -> p j b"),
                  )
                  oht = ohch.tile([128, JP_CHUNK, BC], u8, tag="oht")
                  nc.sync.dma_start(
                      out=oht[:, :cw, :],
                      in_=oh_d[jp0 : jp0 + cw].rearrange("j p b -> p j b"),
                  )
                  eb = eobsch.tile([128, JP_CHUNK, BC], f32, tag="eb")
                  nc.scalar.activation(
                      out=eb[:, :cw, :], in_=ob[:, :cw, :], func=AF.Exp,
                  )
                  eobs_tiles.append(eb)
                  # gold emission partial sums, in GOLD_SUB-wide pieces
                  for g0 in range(0, cw, GOLD_SUB):
                      gw = min(GOLD_SUB, cw - g0)
                      prod = gprod.tile([128, GOLD_SUB, BC], f32, tag="prod")
                      nc.gpsimd.tensor_mul(
                          prod[:, :gw, :], ob[:, g0 : g0 + gw, :], oht[:, g0 : g0 + gw, :]
                      )
                      # free-axis sum on ACT's accumulator port (keeps DVE lean)
                      gpart = gpool.tile([128, 1], f32, tag="gpart")
                      nc.scalar.activation(
                          out=prod[:, :gw, :], in_=prod[:, :gw, :], func=AF.Copy,
                          accum_out=gpart,
                      )
                      nc.gpsimd.tensor_add(gacc_t, gacc_t, gpart)

              # ---- the sequential chain -------------------------------------
              def eobs_slice(j):
                  jj = j - 1
                  parity, jp = jj & 1, jj >> 1
                  c, off = jp // JP_CHUNK, jp % JP_CHUNK
                  return eobs_tiles[c][64 * parity : 64 * parity + K, off, :]

              rescale_idx = {jd: i for i, (jd, _) in enumerate(_rescale_schedule())}
              # two independent 64-column chains: their PE<->DVE ping-pongs
              # overlap, halving the serial per-step latency.
              H = BC // 2
              pending = {}      # apply_step -> bc_psum_tile
              a_prev = [a0A, a0B]
              for j in range(1, NJ + 1):
                  ej_full = eobs_slice(j)
                  psAB = []
                  for h in range(2):
                      ps = pchain.tile([K + 1, H], f32, tag=f"ps{h}")
                      nc.tensor.matmul(
                          ps, lhsT=e_aug, rhs=a_prev[h], start=True, stop=True,
                      )
                      psAB.append(ps)
                  bc_now = pending.pop(j, None)
                  for h in range(2):
                      ej = ej_full[:, h * H : (h + 1) * H]
                      if bc_now is not None:
                          sc = spool.tile([K, H], f32, tag=f"sc{h}")
                          nc.vector.tensor_mul(
                              sc, ej, bc_now[0:K, h * H : (h + 1) * H]
                          )
                          ej = sc
                      a_new = apool.tile([K, H], f32, tag=f"a{h}")
                      nc.vector.tensor_mul(a_new, psAB[h][0:K, :], ej)
                      a_prev[h] = a_new

                  if j in rescale_idx:
                      ri = rescale_idx[j]
                      rt = rpool.tile([128, BC], f32, tag="rt")
                      bc_ps = pmisc.tile([K, BC], f32, tag="bc")
                      for h in range(2):
                          rt_row = rt[64:65, h * H : (h + 1) * H]
                          nc.vector.reciprocal(rt_row, psAB[h][K : K + 1, :])
                          nc.tensor.matmul(
                              bc_ps[0:K, h * H : (h + 1) * H],
                              lhsT=ones_w[64:65, :], rhs=rt_row,
                              start=True, stop=True,
                          )
                          # colsum can reach e^~62; ScalarE Ln domain is
                          # +-2^64. 2^-32 prescale is mantissa-exact; the
                          # +32*ln2 per row is a trace-time constant, undone
                          # in the final bias.
                          nc.scalar.activation(
                              out=lnbuf[:, ri, h * H : (h + 1) * H],
                              in_=psAB[h][K : K + 1, :], func=AF.Ln,
                              scale=float(2.0 ** -32),
                          )
                      pending[j + LAG] = bc_ps

              # ---- endgame ---------------------------------------------------
              # w_b = a_NJ[0,b] + a_NJ[1,b]; logZ = ln(w) + sum_r ln(cs_r)
              # (every emitted rescale was applied to every column).
              w_ps = pmisc.tile([1, BC], f32, tag="scend")
              nc.tensor.matmul(
                  w_ps[:, 0:H], lhsT=e01col, rhs=a_prev[0], start=True, stop=True,
              )
              nc.tensor.matmul(
                  w_ps[:, H:BC], lhsT=e01col, rhs=a_prev[1], start=True, stop=True,
              )
              lnz = endp.tile([1, BC], f32)
              nc.scalar.activation(
                  out=lnz, in_=w_ps, func=AF.Ln, scale=float(2.0 ** -32),
              )
              ssnap = endp.tile([1, BC], f32)
              nc.vector.tensor_reduce(
                  out=ssnap, in_=lnbuf.rearrange("p a b -> p b a"),
                  axis=AX.X, op=ALU.add,
              )
              nc.vector.tensor_add(lnz, lnz, ssnap)
              szl = endp.tile([1, 1], f32)
              nc.vector.tensor_reduce(out=szl, in_=lnz, axis=AX.X, op=ALU.add)
              szl2 = endp.tile([1, 1], f32)
              nc.scalar.activation(
                  out=szl2, in_=szl, func=AF.Copy,
                  bias=float((-1000.0 + (nr + 1) * 32.0 * np.log(2.0)) * BC),
                  scale=1.0,
              )

              ge_ps = pmisc.tile([1, 1], f32, tag="scend")
              nc.tensor.matmul(
                  ge_ps, lhsT=gacc_t, rhs=ones_col, start=True, stop=True,
              )

              gt_ps = pmisc.tile([1, 1], f32, tag="scend")
              nc.tensor.matmul(
                  gt_ps, lhsT=gt, rhs=ones_col[0:K, :], start=True, stop=True,
              )

              fin = endp.tile([1, 4], f32)
              nc.vector.tensor_sub(fin[:, 0:1], szl2, ge_ps)
              nc.vector.tensor_sub(fin[:, 0:1], fin[:, 0:1], gt_ps)
              nc.vector.tensor_copy(out=fin[:, 1:2], in_=szl2)
              nc.vector.tensor_copy(out=fin[:, 2:3], in_=ge_ps)
              nc.vector.tensor_copy(out=fin[:, 3:4], in_=gt_ps)
              nc.sync.dma_start(out=out_d[:, :], in_=fin)

    nc.compile()
    return nc


def _get_program(reps=1):
    if reps not in _PROGRAM_CACHE:
        _PROGRAM_CACHE[reps] = _build_program(reps)
    return _PROGRAM_CACHE[reps]


# --------------------------------------------------------------------------
# entry point
# --------------------------------------------------------------------------

def kernel(pred, ref, seq_len, transitions):
    from concourse import mybir
    from concourse.bass_utils import run_bass_kernel_spmd

    obsP, ohP, cmat, smask = _build_host_tensors(pred, ref, seq_len, np.uint8)
    trans_np = np.ascontiguousarray(
        np.asarray(transitions, dtype=np.float32)[np.ix_(PERM, PERM)])

    nc = _get_program()
    in_maps = [
        {
            "obs": np.ascontiguousarray(obsP[c]),
            "oh": np.ascontiguousarray(ohP[c]),
            "trans": trans_np,
            "cmat": np.ascontiguousarray(cmat[c]),
        }
        for c in range(NCORES)
    ]
    total = np.float64(np.nan)
    for _attempt in range(3):
        res = run_bass_kernel_spmd(
            nc, in_maps, list(range(NCORES)),
            trace=bool(os.environ.get("BASS_TRACE")),
        )
        if res.exec_time_ns is not None:
            print(f"HW exec time: {res.exec_time_ns} ns")
        total = np.float64(0.0)
        for c in range(NCORES):
            total += np.float64(res.results[c]["out"][0, 0])
        if np.isfinite(total):
            break
    return np.array(np.float32(total))



# revision 9
# speedup vs baseline: 1.0971x; 1.0971x over previous
"""CRF loss (nn_CRFLossOld) on 8 Trainium2 NeuronCores.

Forward/backward-split linear-domain CRF:

  - Sequences are sorted by length (desc) and dealt round-robin to the 8
    cores; per core, rank q -> (group g=q%2, slot c=q//2) so both
    partition-halves of each tile carry 64 columns.
  - FORWARD chain (H=258 steps): a_j = (E2f^T a_{j-1}) * exp(obs_j) with
    E2f a 128x128 block-diagonal bf16 stationary (two copies of the
    64x64 exp'd transition matrix; rows 0/1 of each block are the
    keep-alive plumbing, =1). Short sequences (s+1 <= H) complete here
    (extraction at s+1, then keep-alive holds w in rows 0/1).
  - BACKWARD chain (NJB=256 steps) for long sequences, run on the
    e-shifted suffix state bt_j = e_j . beta_j, time-reversed and
    delayed so ALL columns finish at step NJB with beta_H:
      r < r0=513-s: hold [1,1,0..]   r==r0: inject bt_{s+1}=[1,0,..]
      r0<r<NJB: live (global time 513-r)   r==NJB: all-ones e => beta_H
    Stationary M2b = block-diag exp(trans^T) with plumbing rows/cols
    (see _build_program).
  - MERGE: per column Z = sum_k alpha_H[k]*gamma[k]; gamma = beta_H for
    long slots (first W_b free slots), [1,1,0..] for the rest. The dot
    telescopes both chains' per-column rescale shifts.
  - Rescales every R=8 steps per chain: per-group colsum via a tiny
    [128,2] selector matmul, ln on ACT (2^-32 prescale, feeding the
    shift accumulator), 1/cs via ACT exp(-x-32ln2), partition-broadcast
    via a [2,128] matmul, applied LAG=4 steps later by premultiplying
    the exp(obs) arena slice on GPSIMD (SBUF-only operands).
  - Chain arithmetic is bf16 (PE 1 cycle/row; matmuls accumulate fp32
    in PSUM), exact bookkeeping in fp32 shifts.
  - Gold path score: emissions are host-gathered (dtype-preserving
    integer-indexed copy of pred) and summed with one ACT accumulate;
    transitions via per-core integer pair-count matrix dotted with the
    permuted transition table on device.

Host-side prep is integer-derived only: dtype-preserving copies
(packing, gather, transpose), constant fills, and integer counts. All
float arithmetic on pred/transitions happens on-device.
"""

import os
import sys

for _p in ("/opt/trn_rl_repo", "/root/.axon_site/_ro/trn_rl_repo"):
    if os.path.isdir(_p) and _p not in sys.path:
        sys.path.insert(0, _p)

import numpy as np

B, T, L = 1024, 512, 62
K = 64
NCORES = 8
BC = 128                     # sequences per core
H = 258                      # forward steps
NJB = 513 - H + 1            # backward steps: inject + (s-H) live + final
SMALL = -1000.0
R_RESCALE = 8
LAG = 4
CH = 8                       # steps per DMA/exp chunk (small => ACT queue
                             # stays fine-grained; rescale ops don't stall)
LOOKAHEAD = 3                # chunks issued ahead of the chain
LN2_32 = 32.0 * float(np.log(2.0))

# permutation: new index k' -> old label index (0=end, 1=start, 2+l=label l)
PERM = np.concatenate(([63, 62], np.arange(62)))

_PROGRAM_CACHE = {}


# --------------------------------------------------------------------------
# host-side packing (integer-derived only)
# --------------------------------------------------------------------------

def _events(n_steps):
    """Rescale decision steps: 8,16,...,<=n_steps-R-2 (apply must land)."""
    return list(range(R_RESCALE, n_steps - LAG - 1, R_RESCALE))


def _pack(pred, ref, seq_len):
    pred = np.ascontiguousarray(pred, dtype=np.float32)
    ref64 = np.asarray(ref).astype(np.int64)
    s_all = np.asarray(seq_len).astype(np.int64)

    order = np.argsort(-s_all, kind="stable")
    assign = np.empty((NCORES, BC), dtype=np.int64)
    assign[np.arange(B) % NCORES, np.arange(B) // NCORES] = order
    s = s_all[assign]                                   # (C, 128)
    nlong = (s >= H).sum(axis=1)
    W_b = max(1, int(np.ceil(nlong.max() / 2)))

    # ---- forward obs [C, H, 128, 64] -------------------------------------
    obs_f = np.empty((NCORES, H, 128, 64), np.float32)
    obs_f.fill(SMALL)
    jj = np.arange(H)
    for g in (0, 1):
        qs = np.arange(g, BC, 2)
        idx = assign[:, qs]                             # (C, 64)
        sg = s[:, qs]                                   # (C, 64)
        vals = pred[idx, :H, :]                         # (C, 64, H, 62)
        live = jj[None, :, None] < sg[:, None, :]       # (C, H, 64)
        obs_f[:, :, 64 * g + 2 : 64 * g + 64, :] = np.where(
            live[:, :, None, :], vals.transpose(0, 2, 3, 1), np.float32(SMALL)
        )
        # extraction row (k'=0): jj == s  ->  obs 0.0
        ext = jj[None, :, None] == sg[:, None, :]
        obs_f[:, :, 64 * g + 0, :] = np.where(ext, 0.0, SMALL).astype(np.float32)
        # death rows (k'=1): jj > s  ->  keep-alive 0.0
        dead = jj[None, :, None] > sg[:, None, :]
        obs_f[:, :, 64 * g + 1, :] = np.where(dead, 0.0, SMALL).astype(np.float32)

    # ---- backward obs [C, NJB, 128, W_b] ---------------------------------
    obs_b = np.empty((NCORES, NJB, 128, W_b), np.float32)
    obs_b.fill(SMALL)
    rr = np.arange(NJB)                                  # rr = r-1
    for g in (0, 1):
        qs = np.arange(g, 2 * W_b, 2)
        idx = assign[:, qs]                              # (C, W_b)
        sg = s[:, qs]                                    # (C, W_b)
        long = sg >= H                                   # (C, W_b)
        r0 = 513 - sg                                    # valid where long
        r = rr[None, :, None] + 1                        # (1, NJB, 1)
        # emission rows: live where long & r0 < r < NJB; time = 513-r
        liv = long[:, None, :] & (r > r0[:, None, :]) & (r < NJB)
        vals = pred[idx, ::-1, :][:, :, : NJB - 1, :]    # (C, W_b, NJB-1, 62)
        blk = np.where(
            liv[:, : NJB - 1, None, :],
            vals.transpose(0, 2, 3, 1),
            np.float32(SMALL),
        )
        obs_b[:, : NJB - 1, 64 * g + 2 : 64 * g + 64, :] = blk
        # row 0 (end'): 0.0 on hold & inject (r <= r0) or short-hold
        row0 = np.where(long[:, None, :], r <= r0[:, None, :], True)
        obs_b[:, :, 64 * g + 0, :] = np.where(row0, 0.0, SMALL).astype(np.float32)
        # row 1 (keep-alive): 0.0 on hold only (r < r0) or short-hold
        row1 = np.where(long[:, None, :], r < r0[:, None, :], True)
        obs_b[:, :, 64 * g + 1, :] = np.where(row1, 0.0, SMALL).astype(np.float32)
    obs_b[:, NJB - 1, :, :] = 0.0                        # final all-ones step

    # ---- gold emissions (host integer gather, masked) --------------------
    pred_d = pred[assign]                                # (C, 128, T, 62)
    gold = np.take_along_axis(pred_d, ref64[assign][..., None], axis=3)[..., 0]
    tmask = np.arange(T)[None, None, :] < s[:, :, None]
    gold = np.where(tmask, gold, np.float32(0.0)).astype(np.float32)

    # ---- gold transition pair counts (permuted space) --------------------
    cmat = np.zeros((NCORES, K, K), dtype=np.int64)
    ref_d = ref64[assign]                                # (C, 128, T)
    for c in range(NCORES):
        for q in range(BC):
            sq = int(s[c, q])
            path = np.concatenate(([1], ref_d[c, q, :sq] + 2, [0]))
            np.add.at(cmat[c], (path[:-1], path[1:]), 1)

    return obs_f, obs_b, gold, cmat.astype(np.float32), W_b


# --------------------------------------------------------------------------
# device program
# --------------------------------------------------------------------------

class _Chain:
    """One serial chain object: state tile + pools + rescale bookkeeping."""

    def __init__(self, name, w, c0, lhsT, arena, lnbuf, lcol0, apool, ppool):
        self.name = name
        self.w = w                  # free width
        self.c0 = c0                # slot offset inside arena/lnbuf
        self.lhsT = lhsT
        self.arena = arena
        self.lnbuf = lnbuf
        self.lcol0 = lcol0          # column offset inside lnbuf
        self.apool = apool
        self.ppool = ppool
        self.a = None
        self.pending = {}           # apply_step -> bcs sbuf tile


def _build_program(W_b):
    import concourse.bacc as bacc
    import concourse.tile as tile
    from concourse import mybir

    f32 = mybir.dt.float32
    bf16 = mybir.dt.bfloat16
    AF = mybir.ActivationFunctionType
    ALU = mybir.AluOpType
    AX = mybir.AxisListType

    ev_f = _events(H)
    ev_b = _events(NJB)
    nev_f, nev_b = len(ev_f), len(ev_b)

    nc = bacc.Bacc()
    obsf_d = nc.dram_tensor("obsf", [H, 128, 64], f32, kind="ExternalInput")
    obsb_d = nc.dram_tensor("obsb", [NJB, 128, W_b], f32, kind="ExternalInput")
    gold_d = nc.dram_tensor("gold", [128, T], f32, kind="ExternalInput")
    trans_d = nc.dram_tensor("trans", [K, K], f32, kind="ExternalInput")
    transfp_d = nc.dram_tensor("transfp", [K, K], f32, kind="ExternalInput")
    transb_d = nc.dram_tensor("transb", [K, K], f32, kind="ExternalInput")
    bcw_d = nc.dram_tensor("bcwp", [2, 128], f32, kind="ExternalInput")
    cmat_d = nc.dram_tensor("cmat", [K, K], f32, kind="ExternalInput")
    out_d = nc.dram_tensor("out", [1, 8], f32, kind="ExternalOutput")

    with tile.TileContext(nc) as tc:
        with (
            tc.tile_pool(name="const", bufs=1) as const,
            tc.tile_pool(name="stgf", bufs=3) as stgf,
            tc.tile_pool(name="stgb", bufs=3) as stgb,
            tc.tile_pool(name="aF0", bufs=3) as aF0p,
            tc.tile_pool(name="aF1", bufs=3) as aF1p,
            tc.tile_pool(name="aB0", bufs=3) as aB0p,
            tc.tile_pool(name="rsc", bufs=4) as rsc,
            tc.tile_pool(name="endp", bufs=1) as endp,
            tc.tile_pool(name="pF0", bufs=1, space="PSUM") as pF0,
            tc.tile_pool(name="pF1", bufs=1, space="PSUM") as pF1,
            tc.tile_pool(name="pB0", bufs=1, space="PSUM") as pB0,
            tc.tile_pool(name="pmisc", bufs=1, space="PSUM") as pmisc,
        ):
            # ---- constants -----------------------------------------------
            trans_s = const.tile([K, K], f32)
            nc.gpsimd.dma_start(out=trans_s, in_=trans_d[:, :])
            transfp_s = const.tile([K, K], f32)
            nc.gpsimd.dma_start(out=transfp_s, in_=transfp_d[:, :])
            transb_s = const.tile([K, K], f32)
            nc.gpsimd.dma_start(out=transb_s, in_=transb_d[:, :])
            cmat_s = const.tile([K, K], f32)
            nc.gpsimd.dma_start(out=cmat_s, in_=cmat_d[:, :])
            bcw_f32 = const.tile([2, 128], f32)
            nc.gpsimd.dma_start(out=bcw_f32, in_=bcw_d[:, :])

            # stationaries: block-diag exp() of the host-plumbed matrices
            E2f = const.tile([128, 128], bf16)
            nc.vector.memset(E2f, 0.0)
            M2b = const.tile([128, 128], bf16)
            nc.vector.memset(M2b, 0.0)
            for g in (0, 1):
                o = 64 * g
                nc.scalar.activation(
                    out=E2f[o : o + K, o : o + K], in_=transfp_s, func=AF.Exp
                )
                nc.scalar.activation(
                    out=M2b[o : o + K, o : o + K], in_=transb_s, func=AF.Exp
                )

            # per-group colsum selector [128, 2] and broadcast weights [2, 128]
            sel2 = const.tile([128, 2], bf16)
            nc.vector.memset(sel2, 0.0)
            nc.vector.memset(sel2[0:64, 0:1], 1.0)
            nc.vector.memset(sel2[64:128, 1:2], 1.0)
            sel2f = const.tile([128, 2], f32)
            nc.vector.memset(sel2f, 0.0)
            nc.vector.memset(sel2f[0:64, 0:1], 1.0)
            nc.vector.memset(sel2f[64:128, 1:2], 1.0)
            bcw = const.tile([2, 128], bf16)
            nc.scalar.copy(out=bcw, in_=bcw_f32)
            ones_col = const.tile([128, 1], f32)
            nc.vector.memset(ones_col, 1.0)
            nln2_32 = const.tile([2, 1], f32)       # bias tile: -32*ln(2)
            nc.vector.memset(nln2_32, -LN2_32)

            # chain initial states
            a0f = const.tile([128, 64], bf16)
            nc.vector.memset(a0f, 1.0)
            nc.vector.memset(a0f[0:2, :], 0.0)
            nc.vector.memset(a0f[64:66, :], 0.0)
            b0b = const.tile([128, W_b], bf16)
            nc.vector.memset(b0b, 0.0)
            nc.vector.memset(b0b[0:2, :], 1.0)
            nc.vector.memset(b0b[64:66, :], 1.0)

            # shift accumulators (every slot written by schedule)
            lnbuf_f = const.tile([2, nev_f + 1, 64], f32)
            lnbuf_b = const.tile([2, nev_b + 1, W_b], f32)

            # ---- gold terms (early; engines otherwise idle) --------------
            gold_s = const.tile([128, T], f32)
            nc.sync.dma_start(out=gold_s, in_=gold_d[:, :])
            gacc = const.tile([128, 1], f32)
            nc.scalar.activation(
                out=gold_s, in_=gold_s, func=AF.Copy, accum_out=gacc
            )
            trans_st = const.tile([K, K], f32)
            nc.scalar.copy(out=trans_st, in_=trans_s)
            cmat_st = const.tile([K, K], f32)
            nc.scalar.copy(out=cmat_st, in_=cmat_s)
            scr = const.tile([K, K], f32)
            nc.vector.tensor_mul(scr, trans_st, cmat_st)
            gt = const.tile([K, 1], f32)
            nc.vector.tensor_reduce(out=gt, in_=scr, axis=AX.X, op=ALU.add)

            # ---- exp arenas (bf16), streamed in CH-step chunks -----------
            e_f = const.tile([128, H, 64], bf16)
            e_b = const.tile([128, NJB, W_b], bf16)
            nchunk_f = (H + CH - 1) // CH
            nchunk_b = (NJB + CH - 1) // CH

            def issue_chunk_f(ci):
                j0 = ci * CH
                cw = min(CH, H - j0)
                st = stgf.tile([128, CH, 64], f32, tag="stf")
                nc.sync.dma_start(
                    out=st[:, :cw, :],
                    in_=obsf_d[j0 : j0 + cw].rearrange("j p b -> p j b"),
                )
                nc.scalar.activation(
                    out=e_f[:, j0 : j0 + cw, :], in_=st[:, :cw, :], func=AF.Exp
                )

            def issue_chunk_b(ci):
                j0 = ci * CH
                cw = min(CH, NJB - j0)
                st = stgb.tile([128, CH, W_b], f32, tag="stb")
                nc.sync.dma_start(
                    out=st[:, :cw, :],
                    in_=obsb_d[j0 : j0 + cw].rearrange("j p b -> p j b"),
                )
                nc.scalar.activation(
                    out=e_b[:, j0 : j0 + cw, :], in_=st[:, :cw, :], func=AF.Exp
                )

            # ---- chain objects -------------------------------------------
            F0 = _Chain("F0", 32, 0, E2f, e_f, lnbuf_f, 0, aF0p, pF0)
            F1 = _Chain("F1", 32, 32, E2f, e_f, lnbuf_f, 32, aF1p, pF1)
            B0 = _Chain("B0", W_b, 0, M2b, e_b, lnbuf_b, 0, aB0p, pB0)
            F0.a, F1.a = a0f[:, 0:32], a0f[:, 32:64]
            B0.a = b0b
            ev_set_f, ev_set_b = set(ev_f), set(ev_b)

            def chain_step(obj, j, ev_set, ev_list):
                # pending rescale apply: premultiply the arena slice (GPSIMD,
                # SBUF-only) BEFORE this step's mul reads it
                bcs = obj.pending.pop(j, None)
                esl = obj.arena[:, j - 1, obj.c0 : obj.c0 + obj.w]
                if bcs is not None:
                    nc.gpsimd.tensor_mul(esl, esl, bcs)
                ps = obj.ppool.tile([128, obj.w], f32, tag=f"ps{obj.name}")
                nc.tensor.matmul(
                    ps, lhsT=obj.lhsT, rhs=obj.a, start=True, stop=True
                )
                anew = obj.apool.tile([128, obj.w], bf16, tag=f"a{obj.name}")
                nc.vector.tensor_mul(anew, ps, esl)
                obj.a = anew
                if j in ev_set:
                    ri = ev_list.index(j)
                    cs_full = pmisc.tile([2, 64], f32, tag="cs")
                    cs = cs_full[:, : obj.w]
                    nc.tensor.matmul(
                        cs, lhsT=sel2, rhs=anew, start=True, stop=True
                    )
                    lsl = obj.lnbuf[:, ri, obj.lcol0 : obj.lcol0 + obj.w]
                    nc.scalar.activation(
                        out=lsl, in_=cs, func=AF.Ln, scale=float(2.0 ** -32)
                    )
                    bcx = rsc.tile([2, obj.w], bf16, tag="bcx")
                    nc.scalar.activation(
                        out=bcx, in_=lsl, func=AF.Exp, scale=-1.0, bias=nln2_32
                    )
                    psb_full = pmisc.tile([128, 64], f32, tag="psb")
                    psb = psb_full[:, : obj.w]
                    nc.tensor.matmul(
                        psb, lhsT=bcw, rhs=bcx, start=True, stop=True
                    )
                    bcs2 = rsc.tile([128, obj.w], bf16, tag="bcs")
                    nc.scalar.activation(out=bcs2, in_=psb, func=AF.Copy)
                    obj.pending[j + LAG] = bcs2

            # prologue chunks
            nf = nb = 0
            for _ in range(LOOKAHEAD):
                if nf < nchunk_f:
                    issue_chunk_f(nf)
                    nf += 1
                if nb < nchunk_b:
                    issue_chunk_b(nb)
                    nb += 1

            for it in range(1, H + 1):
                while nf < nchunk_f and nf * CH < it + LOOKAHEAD * CH:
                    issue_chunk_f(nf)
                    nf += 1
                while nb < nchunk_b and nb * CH < it + LOOKAHEAD * CH:
                    issue_chunk_b(nb)
                    nb += 1
                chain_step(F0, it, ev_set_f, ev_f)
                chain_step(F1, it, ev_set_f, ev_f)
                if it <= NJB:
                    chain_step(B0, it, ev_set_b, ev_b)

            # ---- endgame --------------------------------------------------
            def final_norm(obj, nev):
                cs_full = pmisc.tile([2, 64], f32, tag="cs")
                cs = cs_full[:, : obj.w]
                nc.tensor.matmul(cs, lhsT=sel2, rhs=obj.a, start=True, stop=True)
                lsl = obj.lnbuf[:, nev, obj.lcol0 : obj.lcol0 + obj.w]
                nc.scalar.activation(
                    out=lsl, in_=cs, func=AF.Ln, scale=float(2.0 ** -32)
                )
                bcx = rsc.tile([2, obj.w], bf16, tag="bcx")
                nc.scalar.activation(
                    out=bcx, in_=lsl, func=AF.Exp, scale=-1.0, bias=nln2_32
                )
                psb_full = pmisc.tile([128, 64], f32, tag="psb")
                psb = psb_full[:, : obj.w]
                nc.tensor.matmul(psb, lhsT=bcw, rhs=bcx, start=True, stop=True)
                an = endp.tile([128, obj.w], f32, tag=f"an{obj.name}")
                nc.vector.tensor_mul(an, psb, obj.a)
                return an

            anF0 = final_norm(F0, nev_f)
            anF1 = final_norm(F1, nev_f)
            anB0 = final_norm(B0, nev_b)

            gm = endp.tile([128, 64], f32)
            nc.vector.memset(gm, 0.0)
            nc.vector.memset(gm[0:2, :], 1.0)
            nc.vector.memset(gm[64:66, :], 1.0)
            nc.vector.tensor_copy(out=gm[:, 0:W_b], in_=anB0)

            m0 = endp.tile([128, 64], f32)
            nc.vector.tensor_mul(m0[:, 0:32], anF0, gm[:, 0:32])
            nc.vector.tensor_mul(m0[:, 32:64], anF1, gm[:, 32:64])
            psd = pmisc.tile([2, 64], f32, tag="cs")
            nc.tensor.matmul(psd, lhsT=sel2f, rhs=m0, start=True, stop=True)
            lnd = endp.tile([2, 64], f32)
            nc.scalar.activation(out=lnd, in_=psd, func=AF.Ln)

            ssf = endp.tile([2, 64], f32)
            nc.vector.tensor_reduce(
                out=ssf, in_=lnbuf_f.rearrange("p a b -> p b a"),
                axis=AX.X, op=ALU.add,
            )
            ssb = endp.tile([2, W_b], f32)
            nc.vector.tensor_reduce(
                out=ssb, in_=lnbuf_b.rearrange("p a b -> p b a"),
                axis=AX.X, op=ALU.add,
            )
            tot = endp.tile([2, 64], f32)
            nc.vector.tensor_add(tot, lnd, ssf)
            nc.vector.tensor_add(tot[:, 0:W_b], tot[:, 0:W_b], ssb)
            red = endp.tile([2, 1], f32)
            nc.vector.tensor_reduce(out=red, in_=tot, axis=AX.X, op=ALU.add)
            ones2 = endp.tile([2, 1], f32)
            nc.vector.memset(ones2, 1.0)
            zps = pmisc.tile([1, 1], f32, tag="sc1")
            nc.tensor.matmul(zps, lhsT=red, rhs=ones2, start=True, stop=True)
            bias = (
                BC * (LN2_32 * (nev_f + 1) - 1000.0)
                + 2.0 * W_b * LN2_32 * (nev_b + 1)
            )
            szl = endp.tile([1, 1], f32)
            nc.scalar.activation(
                out=szl, in_=zps, func=AF.Copy, bias=float(bias), scale=1.0
            )

            ge_ps = pmisc.tile([1, 1], f32, tag="sc1")
            nc.tensor.matmul(
                ge_ps, lhsT=gacc, rhs=ones_col, start=True, stop=True
            )
            gesb = endp.tile([1, 1], f32)
            nc.vector.tensor_copy(out=gesb, in_=ge_ps)
            gt_ps = pmisc.tile([1, 1], f32, tag="sc1")
            nc.tensor.matmul(
                gt_ps, lhsT=gt, rhs=ones_col[0:K, :], start=True, stop=True
            )

            fin = endp.tile([1, 8], f32)
            nc.vector.tensor_sub(fin[:, 0:1], szl, gesb)
            nc.vector.tensor_sub(fin[:, 0:1], fin[:, 0:1], gt_ps)
            nc.vector.tensor_copy(out=fin[:, 1:2], in_=szl)
            nc.vector.tensor_copy(out=fin[:, 2:3], in_=gesb)
            nc.vector.tensor_copy(out=fin[:, 3:4], in_=gt_ps)
            nc.vector.memset(fin[:, 4:8], 0.0)
            nc.sync.dma_start(out=out_d[:, :], in_=fin)

    nc.compile()
    return nc


def _get_program(W_b):
    if W_b not in _PROGRAM_CACHE:
        _PROGRAM_CACHE[W_b] = _build_program(W_b)
    return _PROGRAM_CACHE[W_b]


# --------------------------------------------------------------------------
# entry point
# --------------------------------------------------------------------------

def kernel(pred, ref, seq_len, transitions):
    from concourse.bass_utils import run_bass_kernel_spmd

    obs_f, obs_b, gold, cmat, W_b = _pack(pred, ref, seq_len)
    trans_f = np.ascontiguousarray(
        np.asarray(transitions, dtype=np.float32)[np.ix_(PERM, PERM)]
    )
    # plumbing as constant fills (mirrors the reference's own -10000 fills):
    # fwd: rows 0/1 (from end/keep-alive) -> 0.0 so exp()=1 keep-alive rows
    transf_p = trans_f.copy()
    transf_p[0, :] = 0.0
    transf_p[1, :] = 0.0
    # bwd stationary (lhsT = E^T with hold/inject plumbing):
    #   row1 = e1 (hold), [0,0]=1, [0,1]=0; rows>=2 cols 0/1 already -10000
    transb_p = np.ascontiguousarray(trans_f.T)
    transb_p[1, :] = -10000.0
    transb_p[1, 1] = 0.0
    transb_p[0, 0] = 0.0
    transb_p[0, 1] = -10000.0
    # partition-broadcast stationary pattern [2, 128]
    bcw_np = np.zeros((2, 128), np.float32)
    bcw_np[0, 0:64] = 1.0
    bcw_np[1, 64:128] = 1.0

    nc = _get_program(W_b)
    in_maps = [
        {
            "obsf": np.ascontiguousarray(obs_f[c]),
            "obsb": np.ascontiguousarray(obs_b[c]),
            "gold": np.ascontiguousarray(gold[c]),
            "trans": trans_f,
            "transfp": transf_p,
            "transb": transb_p,
            "bcwp": bcw_np,
            "cmat": np.ascontiguousarray(cmat[c]),
        }
        for c in range(NCORES)
    ]
    total = np.float64(np.nan)
    for _attempt in range(3):
        res = run_bass_kernel_spmd(
            nc, in_maps, list(range(NCORES)),
            trace=bool(os.environ.get("BASS_TRACE")),
        )
        if res.exec_time_ns is not None:
            print(f"HW exec time: {res.exec_time_ns} ns")
        total = np.float64(0.0)
        for c in range(NCORES):
            total += np.float64(res.results[c]["out"][0, 0])
        if np.isfinite(total):
            break
    return np.array(np.float32(total))


# revision 12
# speedup vs baseline: 2.5752x; 2.3473x over previous
"""CRF loss (nn_CRFLossOld) on 8 Trainium2 NeuronCores.

Forward/backward-split linear-domain CRF:

  - Sequences are sorted by length (desc) and dealt round-robin to the 8
    cores; per core, rank q -> (group g=q%2, slot c=q//2) so both
    partition-halves of each tile carry 64 columns.
  - FORWARD chain (H=258 steps): a_j = (E2f^T a_{j-1}) * exp(obs_j) with
    E2f a 128x128 block-diagonal bf16 stationary (two copies of the
    64x64 exp'd transition matrix; rows 0/1 of each block are the
    keep-alive plumbing, =1). Short sequences (s+1 <= H) complete here
    (extraction at s+1, then keep-alive holds w in rows 0/1).
  - BACKWARD chain (NJB=256 steps) for long sequences, run on the
    e-shifted suffix state bt_j = e_j . beta_j, time-reversed and
    delayed so ALL columns finish at step NJB with beta_H:
      r < r0=513-s: hold [1,1,0..]   r==r0: inject bt_{s+1}=[1,0,..]
      r0<r<NJB: live (global time 513-r)   r==NJB: all-ones e => beta_H
    Stationary M2b = block-diag exp(trans^T) with plumbing rows/cols
    (see _build_program).
  - MERGE: per column Z = sum_k alpha_H[k]*gamma[k]; gamma = beta_H for
    long slots (first W_b free slots), [1,1,0..] for the rest. The dot
    telescopes both chains' per-column rescale shifts.
  - Rescales every R=8 steps per chain: per-group colsum via a tiny
    [128,2] selector matmul, ln on ACT (2^-32 prescale, feeding the
    shift accumulator), 1/cs via ACT exp(-x-32ln2), partition-broadcast
    via a [2,128] matmul, applied LAG=4 steps later by premultiplying
    the exp(obs) arena slice on GPSIMD (SBUF-only operands).
  - Chain arithmetic is bf16 (PE 1 cycle/row; matmuls accumulate fp32
    in PSUM), exact bookkeeping in fp32 shifts.
  - Gold path score: emissions are host-gathered (dtype-preserving
    integer-indexed copy of pred) and summed with one ACT accumulate;
    transitions via per-core integer pair-count matrix dotted with the
    permuted transition table on device.

Host-side prep is integer-derived only: dtype-preserving copies
(packing, gather, transpose), constant fills, and integer counts. All
float arithmetic on pred/transitions happens on-device.
"""

import os
import sys

for _p in ("/opt/trn_rl_repo", "/root/.axon_site/_ro/trn_rl_repo"):
    if os.path.isdir(_p) and _p not in sys.path:
        sys.path.insert(0, _p)

import numpy as np

B, T, L = 1024, 512, 62
K = 64
NCORES = 8
BC = 128                     # sequences per core
H = 258                      # forward steps
NJB = 513 - H + 1            # backward steps: inject + (s-H) live + final
SMALL = -1000.0
R_RESCALE = 8
LAG = 4
CH = 8                       # steps per DMA/exp chunk (small => ACT queue
                             # stays fine-grained; rescale ops don't stall)
LOOKAHEAD = 3                # chunks issued ahead of the chain
LN2_32 = 32.0 * float(np.log(2.0))

# permutation: new index k' -> old label index (0=end, 1=start, 2+l=label l)
PERM = np.concatenate(([63, 62], np.arange(62)))

_PROGRAM_CACHE = {}


# --------------------------------------------------------------------------
# host-side packing (integer-derived only)
# --------------------------------------------------------------------------

def _events(n_steps):
    """Rescale decision steps: 8,16,...,<=n_steps-R-2 (apply must land)."""
    return list(range(R_RESCALE, n_steps - LAG - 1, R_RESCALE))


def _pack(pred, ref, seq_len):
    pred = np.ascontiguousarray(pred, dtype=np.float32)
    ref64 = np.asarray(ref).astype(np.int64)
    s_all = np.asarray(seq_len).astype(np.int64)

    order = np.argsort(-s_all, kind="stable")
    assign = np.empty((NCORES, BC), dtype=np.int64)
    assign[np.arange(B) % NCORES, np.arange(B) // NCORES] = order
    s = s_all[assign]                                   # (C, 128)
    nlong = (s >= H).sum(axis=1)
    W_b = max(1, int(np.ceil(nlong.max() / 2)))

    # ---- forward obs [C, H, 128, 64] -------------------------------------
    obs_f = np.empty((NCORES, H, 128, 64), np.float32)
    obs_f.fill(SMALL)
    jj = np.arange(H)
    for g in (0, 1):
        qs = np.arange(g, BC, 2)
        idx = assign[:, qs]                             # (C, 64)
        sg = s[:, qs]                                   # (C, 64)
        vals = pred[idx, :H, :]                         # (C, 64, H, 62)
        live = jj[None, :, None] < sg[:, None, :]       # (C, H, 64)
        obs_f[:, :, 64 * g + 2 : 64 * g + 64, :] = np.where(
            live[:, :, None, :], vals.transpose(0, 2, 3, 1), np.float32(SMALL)
        )
        # extraction row (k'=0): jj == s  ->  obs 0.0
        ext = jj[None, :, None] == sg[:, None, :]
        obs_f[:, :, 64 * g + 0, :] = np.where(ext, 0.0, SMALL).astype(np.float32)
        # death rows (k'=1): jj > s  ->  keep-alive 0.0
        dead = jj[None, :, None] > sg[:, None, :]
        obs_f[:, :, 64 * g + 1, :] = np.where(dead, 0.0, SMALL).astype(np.float32)

    # ---- backward obs [C, NJB, 128, W_b] ---------------------------------
    obs_b = np.empty((NCORES, NJB, 128, W_b), np.float32)
    obs_b.fill(SMALL)
    rr = np.arange(NJB)                                  # rr = r-1
    for g in (0, 1):
        qs = np.arange(g, 2 * W_b, 2)
        idx = assign[:, qs]                              # (C, W_b)
        sg = s[:, qs]                                    # (C, W_b)
        long = sg >= H                                   # (C, W_b)
        r0 = 513 - sg                                    # valid where long
        r = rr[None, :, None] + 1                        # (1, NJB, 1)
        # emission rows: live where long & r0 < r < NJB; time = 513-r
        liv = long[:, None, :] & (r > r0[:, None, :]) & (r < NJB)
        vals = pred[idx, ::-1, :][:, :, : NJB - 1, :]    # (C, W_b, NJB-1, 62)
        blk = np.where(
            liv[:, : NJB - 1, None, :],
            vals.transpose(0, 2, 3, 1),
            np.float32(SMALL),
        )
        obs_b[:, : NJB - 1, 64 * g + 2 : 64 * g + 64, :] = blk
        # row 0 (end'): 0.0 on hold & inject (r <= r0) or short-hold
        row0 = np.where(long[:, None, :], r <= r0[:, None, :], True)
        obs_b[:, :, 64 * g + 0, :] = np.where(row0, 0.0, SMALL).astype(np.float32)
        # row 1 (keep-alive): 0.0 on hold only (r < r0) or short-hold
        row1 = np.where(long[:, None, :], r < r0[:, None, :], True)
        obs_b[:, :, 64 * g + 1, :] = np.where(row1, 0.0, SMALL).astype(np.float32)
    obs_b[:, NJB - 1, :, :] = 0.0                        # final all-ones step

    # ---- gold emissions (host integer gather, masked) --------------------
    pred_d = pred[assign]                                # (C, 128, T, 62)
    gold = np.take_along_axis(pred_d, ref64[assign][..., None], axis=3)[..., 0]
    tmask = np.arange(T)[None, None, :] < s[:, :, None]
    gold = np.where(tmask, gold, np.float32(0.0)).astype(np.float32)

    # ---- gold transition pair counts (permuted space) --------------------
    cmat = np.zeros((NCORES, K, K), dtype=np.int64)
    ref_d = ref64[assign]                                # (C, 128, T)
    for c in range(NCORES):
        for q in range(BC):
            sq = int(s[c, q])
            path = np.concatenate(([1], ref_d[c, q, :sq] + 2, [0]))
            np.add.at(cmat[c], (path[:-1], path[1:]), 1)

    return obs_f, obs_b, gold, cmat.astype(np.float32), W_b


# --------------------------------------------------------------------------
# device program
# --------------------------------------------------------------------------

class _Chain:
    """One serial chain object: state tile + pools + rescale bookkeeping."""

    def __init__(self, name, w, c0, lhsT, arena, lnbuf, lcol0, apool, ppool):
        self.name = name
        self.w = w                  # free width
        self.c0 = c0                # slot offset inside arena/lnbuf
        self.lhsT = lhsT
        self.arena = arena
        self.lnbuf = lnbuf
        self.lcol0 = lcol0          # column offset inside lnbuf
        self.apool = apool
        self.ppool = ppool
        self.a = None
        self.pending = {}           # apply_step -> bcs sbuf tile


def _build_program(W_b):
    import concourse.bacc as bacc
    import concourse.tile as tile
    from concourse import mybir

    f32 = mybir.dt.float32
    bf16 = mybir.dt.bfloat16
    AF = mybir.ActivationFunctionType
    ALU = mybir.AluOpType
    AX = mybir.AxisListType

    ev_f = _events(H)
    ev_b = _events(NJB)
    nev_f, nev_b = len(ev_f), len(ev_b)

    nc = bacc.Bacc()
    obsf_d = nc.dram_tensor("obsf", [H, 128, 64], f32, kind="ExternalInput")
    obsb_d = nc.dram_tensor("obsb", [NJB, 128, W_b], f32, kind="ExternalInput")
    gold_d = nc.dram_tensor("gold", [128, T], f32, kind="ExternalInput")
    trans_d = nc.dram_tensor("trans", [K, K], f32, kind="ExternalInput")
    transfp_d = nc.dram_tensor("transfp", [K, K], f32, kind="ExternalInput")
    transb_d = nc.dram_tensor("transb", [K, K], f32, kind="ExternalInput")
    bcw_d = nc.dram_tensor("bcwp", [2, 128], f32, kind="ExternalInput")
    cmat_d = nc.dram_tensor("cmat", [K, K], f32, kind="ExternalInput")
    out_d = nc.dram_tensor("out", [1, 8], f32, kind="ExternalOutput")

    with tile.TileContext(nc) as tc:
        with (
            tc.tile_pool(name="const", bufs=1) as const,
            tc.tile_pool(name="stgf", bufs=3) as stgf,
            tc.tile_pool(name="stgb", bufs=3) as stgb,
            tc.tile_pool(name="aF0", bufs=3) as aF0p,
            tc.tile_pool(name="aF1", bufs=3) as aF1p,
            tc.tile_pool(name="aB0", bufs=3) as aB0p,
            tc.tile_pool(name="rsc", bufs=4) as rsc,
            tc.tile_pool(name="endp", bufs=1) as endp,
            tc.tile_pool(name="pF0", bufs=1, space="PSUM") as pF0,
            tc.tile_pool(name="pF1", bufs=1, space="PSUM") as pF1,
            tc.tile_pool(name="pB0", bufs=1, space="PSUM") as pB0,
            tc.tile_pool(name="pmisc", bufs=1, space="PSUM") as pmisc,
        ):
            # ---- constants -----------------------------------------------
            trans_s = const.tile([K, K], f32)
            nc.gpsimd.dma_start(out=trans_s, in_=trans_d[:, :])
            transfp_s = const.tile([K, K], f32)
            nc.gpsimd.dma_start(out=transfp_s, in_=transfp_d[:, :])
            transb_s = const.tile([K, K], f32)
            nc.gpsimd.dma_start(out=transb_s, in_=transb_d[:, :])
            cmat_s = const.tile([K, K], f32)
            nc.gpsimd.dma_start(out=cmat_s, in_=cmat_d[:, :])
            bcw_f32 = const.tile([2, 128], f32)
            nc.gpsimd.dma_start(out=bcw_f32, in_=bcw_d[:, :])

            # stationaries: block-diag exp() of the host-plumbed matrices
            E2f = const.tile([128, 128], bf16)
            nc.vector.memset(E2f, 0.0)
            M2b = const.tile([128, 128], bf16)
            nc.vector.memset(M2b, 0.0)
            for g in (0, 1):
                o = 64 * g
                nc.scalar.activation(
                    out=E2f[o : o + K, o : o + K], in_=transfp_s, func=AF.Exp
                )
                nc.scalar.activation(
                    out=M2b[o : o + K, o : o + K], in_=transb_s, func=AF.Exp
                )

            # per-group colsum selector [128, 2] and broadcast weights [2, 128]
            sel2 = const.tile([128, 2], bf16)
            nc.vector.memset(sel2, 0.0)
            nc.vector.memset(sel2[0:64, 0:1], 1.0)
            nc.vector.memset(sel2[64:128, 1:2], 1.0)
            sel2f = const.tile([128, 2], f32)
            nc.vector.memset(sel2f, 0.0)
            nc.vector.memset(sel2f[0:64, 0:1], 1.0)
            nc.vector.memset(sel2f[64:128, 1:2], 1.0)
            bcw = const.tile([2, 128], bf16)
            nc.scalar.copy(out=bcw, in_=bcw_f32)
            ones_col = const.tile([128, 1], f32)
            nc.vector.memset(ones_col, 1.0)

            # chain initial states
            a0f = const.tile([128, 64], bf16)
            nc.vector.memset(a0f, 1.0)
            nc.vector.memset(a0f[0:2, :], 0.0)
            nc.vector.memset(a0f[64:66, :], 0.0)
            b0b = const.tile([128, W_b], bf16)
            nc.vector.memset(b0b, 0.0)
            nc.vector.memset(b0b[0:2, :], 1.0)
            nc.vector.memset(b0b[64:66, :], 1.0)

            # raw per-event colsum stash (ln'd in one batched op at endgame)
            csbuf_f = const.tile([2, nev_f + 1, 64], f32)
            csbuf_b = const.tile([2, nev_b + 1, W_b], f32)

            # ---- gold terms (early; engines otherwise idle) --------------
            gold_s = const.tile([128, T], f32)
            nc.sync.dma_start(out=gold_s, in_=gold_d[:, :])
            gacc = const.tile([128, 1], f32)
            nc.scalar.activation(
                out=gold_s, in_=gold_s, func=AF.Copy, accum_out=gacc
            )
            trans_st = const.tile([K, K], f32)
            nc.scalar.copy(out=trans_st, in_=trans_s)
            cmat_st = const.tile([K, K], f32)
            nc.scalar.copy(out=cmat_st, in_=cmat_s)
            scr = const.tile([K, K], f32)
            nc.vector.tensor_mul(scr, trans_st, cmat_st)
            gt = const.tile([K, 1], f32)
            nc.vector.tensor_reduce(out=gt, in_=scr, axis=AX.X, op=ALU.add)

            # ---- exp arenas (bf16), streamed in CH-step chunks -----------
            e_f = const.tile([128, H, 64], bf16)
            e_b = const.tile([128, NJB, W_b], bf16)
            nchunk_f = (H + CH - 1) // CH
            nchunk_b = (NJB + CH - 1) // CH

            def issue_chunk_f(ci):
                j0 = ci * CH
                cw = min(CH, H - j0)
                st = stgf.tile([128, CH, 64], f32, tag="stf")
                nc.sync.dma_start(
                    out=st[:, :cw, :],
                    in_=obsf_d[j0 : j0 + cw].rearrange("j p b -> p j b"),
                )
                nc.scalar.activation(
                    out=e_f[:, j0 : j0 + cw, :], in_=st[:, :cw, :], func=AF.Exp
                )

            def issue_chunk_b(ci):
                j0 = ci * CH
                cw = min(CH, NJB - j0)
                st = stgb.tile([128, CH, W_b], f32, tag="stb")
                nc.sync.dma_start(
                    out=st[:, :cw, :],
                    in_=obsb_d[j0 : j0 + cw].rearrange("j p b -> p j b"),
                )
                nc.scalar.activation(
                    out=e_b[:, j0 : j0 + cw, :], in_=st[:, :cw, :], func=AF.Exp
                )

            # ---- chain objects -------------------------------------------
            F0 = _Chain("F0", 32, 0, E2f, e_f, csbuf_f, 0, aF0p, pF0)
            F1 = _Chain("F1", 32, 32, E2f, e_f, csbuf_f, 32, aF1p, pF1)
            B0 = _Chain("B0", W_b, 0, M2b, e_b, csbuf_b, 0, aB0p, pB0)
            F0.a, F1.a = a0f[:, 0:32], a0f[:, 32:64]
            B0.a = b0b
            ev_set_f, ev_set_b = set(ev_f), set(ev_b)

            def chain_step(obj, j, ev_set, ev_list):
                # pending rescale apply: premultiply the arena slice (GPSIMD,
                # SBUF-only) BEFORE this step's mul reads it
                bcs = obj.pending.pop(j, None)
                esl = obj.arena[:, j - 1, obj.c0 : obj.c0 + obj.w]
                if bcs is not None:
                    nc.gpsimd.tensor_mul(esl, esl, bcs)
                ps = obj.ppool.tile([128, obj.w], f32, tag=f"ps{obj.name}")
                nc.tensor.matmul(
                    ps, lhsT=obj.lhsT, rhs=obj.a, start=True, stop=True
                )
                anew = obj.apool.tile([128, obj.w], bf16, tag=f"a{obj.name}")
                nc.vector.tensor_mul(anew, ps, esl)
                obj.a = anew
                if j in ev_set:
                    ri = ev_list.index(j)
                    cs_full = pmisc.tile([2, 64], f32, tag="cs")
                    cs = cs_full[:, : obj.w]
                    nc.tensor.matmul(
                        cs, lhsT=sel2, rhs=anew, start=True, stop=True
                    )
                    csl = obj.lnbuf[:, ri, obj.lcol0 : obj.lcol0 + obj.w]
                    nc.scalar.activation(out=csl, in_=cs, func=AF.Copy)
                    bcx = rsc.tile([2, obj.w], bf16, tag="bcx")
                    with nc.allow_low_precision(reason="rescale factor; drift lands in tracked shifts"):
                        nc.vector.reciprocal(bcx, cs)
                    psb_full = pmisc.tile([128, 64], f32, tag="psb")
                    psb = psb_full[:, : obj.w]
                    nc.tensor.matmul(
                        psb, lhsT=bcw, rhs=bcx, start=True, stop=True
                    )
                    bcs2 = rsc.tile([128, obj.w], bf16, tag="bcs")
                    nc.scalar.activation(out=bcs2, in_=psb, func=AF.Copy)
                    obj.pending[j + LAG] = bcs2

            # prologue chunks
            nf = nb = 0
            for _ in range(LOOKAHEAD):
                if nf < nchunk_f:
                    issue_chunk_f(nf)
                    nf += 1
                if nb < nchunk_b:
                    issue_chunk_b(nb)
                    nb += 1

            for it in range(1, H + 1):
                while nf < nchunk_f and nf * CH < it + LOOKAHEAD * CH:
                    issue_chunk_f(nf)
                    nf += 1
                while nb < nchunk_b and nb * CH < it + LOOKAHEAD * CH:
                    issue_chunk_b(nb)
                    nb += 1
                chain_step(F0, it, ev_set_f, ev_f)
                chain_step(F1, it, ev_set_f, ev_f)
                if it <= NJB:
                    chain_step(B0, it, ev_set_b, ev_b)

            # ---- endgame --------------------------------------------------
            def final_norm(obj, nev):
                cs_full = pmisc.tile([2, 64], f32, tag="cs")
                cs = cs_full[:, : obj.w]
                nc.tensor.matmul(cs, lhsT=sel2, rhs=obj.a, start=True, stop=True)
                csl = obj.lnbuf[:, nev, obj.lcol0 : obj.lcol0 + obj.w]
                nc.scalar.activation(out=csl, in_=cs, func=AF.Copy)
                bcx = rsc.tile([2, obj.w], bf16, tag="bcx")
                with nc.allow_low_precision(reason="rescale factor; drift lands in tracked shifts"):
                    nc.vector.reciprocal(bcx, cs)
                psb_full = pmisc.tile([128, 64], f32, tag="psb")
                psb = psb_full[:, : obj.w]
                nc.tensor.matmul(psb, lhsT=bcw, rhs=bcx, start=True, stop=True)
                an = endp.tile([128, obj.w], f32, tag=f"an{obj.name}")
                nc.vector.tensor_mul(an, psb, obj.a)
                return an

            anF0 = final_norm(F0, nev_f)
            anF1 = final_norm(F1, nev_f)
            anB0 = final_norm(B0, nev_b)

            gm = endp.tile([128, 64], f32)
            nc.vector.memset(gm, 0.0)
            nc.vector.memset(gm[0:2, :], 1.0)
            nc.vector.memset(gm[64:66, :], 1.0)
            nc.vector.tensor_copy(out=gm[:, 0:W_b], in_=anB0)

            m0 = endp.tile([128, 64], f32)
            nc.vector.tensor_mul(m0[:, 0:32], anF0, gm[:, 0:32])
            nc.vector.tensor_mul(m0[:, 32:64], anF1, gm[:, 32:64])
            psd = pmisc.tile([2, 64], f32, tag="cs")
            nc.tensor.matmul(psd, lhsT=sel2f, rhs=m0, start=True, stop=True)
            lnd = endp.tile([2, 64], f32)
            nc.scalar.activation(out=lnd, in_=psd, func=AF.Ln)

            # 2^-32 prescale keeps the Ln arg inside ScalarE's +-2^64 domain;
            # the +32ln2 per event is folded into the final bias constant
            lnbuf_f = endp.tile([2, nev_f + 1, 64], f32)
            nc.scalar.activation(
                out=lnbuf_f, in_=csbuf_f, func=AF.Ln, scale=float(2.0 ** -32)
            )
            lnbuf_b = endp.tile([2, nev_b + 1, W_b], f32)
            nc.scalar.activation(
                out=lnbuf_b, in_=csbuf_b, func=AF.Ln, scale=float(2.0 ** -32)
            )
            ssf = endp.tile([2, 64], f32)
            nc.vector.tensor_reduce(
                out=ssf, in_=lnbuf_f.rearrange("p a b -> p b a"),
                axis=AX.X, op=ALU.add,
            )
            ssb = endp.tile([2, W_b], f32)
            nc.vector.tensor_reduce(
                out=ssb, in_=lnbuf_b.rearrange("p a b -> p b a"),
                axis=AX.X, op=ALU.add,
            )
            tot = endp.tile([2, 64], f32)
            nc.vector.tensor_add(tot, lnd, ssf)
            nc.vector.tensor_add(tot[:, 0:W_b], tot[:, 0:W_b], ssb)
            red = endp.tile([2, 1], f32)
            nc.vector.tensor_reduce(out=red, in_=tot, axis=AX.X, op=ALU.add)
            ones2 = endp.tile([2, 1], f32)
            nc.vector.memset(ones2, 1.0)
            zps = pmisc.tile([1, 1], f32, tag="sc1")
            nc.tensor.matmul(zps, lhsT=red, rhs=ones2, start=True, stop=True)
            bias = (
                BC * (LN2_32 * (nev_f + 1) - 1000.0)
                + 2.0 * W_b * LN2_32 * (nev_b + 1)
            )
            szl = endp.tile([1, 1], f32)
            nc.scalar.activation(
                out=szl, in_=zps, func=AF.Copy, bias=float(bias), scale=1.0
            )

            ge_ps = pmisc.tile([1, 1], f32, tag="sc1")
            nc.tensor.matmul(
                ge_ps, lhsT=gacc, rhs=ones_col, start=True, stop=True
            )
            gesb = endp.tile([1, 1], f32)
            nc.vector.tensor_copy(out=gesb, in_=ge_ps)
            gt_ps = pmisc.tile([1, 1], f32, tag="sc1")
            nc.tensor.matmul(
                gt_ps, lhsT=gt, rhs=ones_col[0:K, :], start=True, stop=True
            )

            fin = endp.tile([1, 8], f32)
            nc.vector.tensor_sub(fin[:, 0:1], szl, gesb)
            nc.vector.tensor_sub(fin[:, 0:1], fin[:, 0:1], gt_ps)
            nc.vector.tensor_copy(out=fin[:, 1:2], in_=szl)
            nc.vector.tensor_copy(out=fin[:, 2:3], in_=gesb)
            nc.vector.tensor_copy(out=fin[:, 3:4], in_=gt_ps)
            nc.vector.memset(fin[:, 4:8], 0.0)
            nc.sync.dma_start(out=out_d[:, :], in_=fin)

    nc.compile()
    return nc


def _get_program(W_b):
    if W_b not in _PROGRAM_CACHE:
        _PROGRAM_CACHE[W_b] = _build_program(W_b)
    return _PROGRAM_CACHE[W_b]


# --------------------------------------------------------------------------
# entry point
# --------------------------------------------------------------------------

def kernel(pred, ref, seq_len, transitions):
    from concourse.bass_utils import run_bass_kernel_spmd

    obs_f, obs_b, gold, cmat, W_b = _pack(pred, ref, seq_len)
    trans_f = np.ascontiguousarray(
        np.asarray(transitions, dtype=np.float32)[np.ix_(PERM, PERM)]
    )
    # plumbing as constant fills (mirrors the reference's own -10000 fills):
    # fwd: rows 0/1 (from end/keep-alive) -> 0.0 so exp()=1 keep-alive rows
    transf_p = trans_f.copy()
    transf_p[0, :] = 0.0
    transf_p[1, :] = 0.0
    # bwd stationary (lhsT = E^T with hold/inject plumbing):
    #   row1 = e1 (hold), [0,0]=1, [0,1]=0; rows>=2 cols 0/1 already -10000
    transb_p = np.ascontiguousarray(trans_f.T)
    transb_p[1, :] = -10000.0
    transb_p[1, 1] = 0.0
    transb_p[0, 0] = 0.0
    transb_p[0, 1] = -10000.0
    # partition-broadcast stationary pattern [2, 128]
    bcw_np = np.zeros((2, 128), np.float32)
    bcw_np[0, 0:64] = 1.0
    bcw_np[1, 64:128] = 1.0

    nc = _get_program(W_b)
    in_maps = [
        {
            "obsf": np.ascontiguousarray(obs_f[c]),
            "obsb": np.ascontiguousarray(obs_b[c]),
            "gold": np.ascontiguousarray(gold[c]),
            "trans": trans_f,
            "transfp": transf_p,
            "transb": transb_p,
            "bcwp": bcw_np,
            "cmat": np.ascontiguousarray(cmat[c]),
        }
        for c in range(NCORES)
    ]
    total = np.float64(np.nan)
    for _attempt in range(3):
        res = run_bass_kernel_spmd(
            nc, in_maps, list(range(NCORES)),
            trace=bool(os.environ.get("BASS_TRACE")),
        )
        if res.exec_time_ns is not None:
            print(f"HW exec time: {res.exec_time_ns} ns")
        total = np.float64(0.0)
        for c in range(NCORES):
            total += np.float64(res.results[c]["out"][0, 0])
        if np.isfinite(total):
            break
    return np.array(np.float32(total))


# revision 15
# speedup vs baseline: 3.0378x; 1.1797x over previous
"""CRF loss (nn_CRFLossOld) on 8 Trainium2 NeuronCores.

Forward/backward-split linear-domain CRF:

  - Sequences are sorted by length (desc) and dealt round-robin to the 8
    cores; per core, rank q -> (group g=q%2, slot c=q//2) so both
    partition-halves of each tile carry 64 columns.
  - FORWARD chain (H=258 steps): a_j = (E2f^T a_{j-1}) * exp(obs_j) with
    E2f a 128x128 block-diagonal bf16 stationary (two copies of the
    64x64 exp'd transition matrix; rows 0/1 of each block are the
    keep-alive plumbing, =1). Short sequences (s+1 <= H) complete here
    (extraction at s+1, then keep-alive holds w in rows 0/1).
  - BACKWARD chain (NJB=256 steps) for long sequences, run on the
    e-shifted suffix state bt_j = e_j . beta_j, time-reversed and
    delayed so ALL columns finish at step NJB with beta_H:
      r < r0=513-s: hold [1,1,0..]   r==r0: inject bt_{s+1}=[1,0,..]
      r0<r<NJB: live (global time 513-r)   r==NJB: all-ones e => beta_H
    Stationary M2b = block-diag exp(trans^T) with plumbing rows/cols
    (see _build_program).
  - MERGE: per column Z = sum_k alpha_H[k]*gamma[k]; gamma = beta_H for
    long slots (first W_b free slots), [1,1,0..] for the rest. The dot
    telescopes both chains' per-column rescale shifts.
  - Rescales every R=8 steps per chain: per-group colsum via a tiny
    [128,2] selector matmul, ln on ACT (2^-32 prescale, feeding the
    shift accumulator), 1/cs via ACT exp(-x-32ln2), partition-broadcast
    via a [2,128] matmul, applied LAG=4 steps later by premultiplying
    the exp(obs) arena slice on GPSIMD (SBUF-only operands).
  - Chain arithmetic is bf16 (PE 1 cycle/row; matmuls accumulate fp32
    in PSUM), exact bookkeeping in fp32 shifts.
  - Gold path score: emissions are host-gathered (dtype-preserving
    integer-indexed copy of pred) and summed with one ACT accumulate;
    transitions via per-core integer pair-count matrix dotted with the
    permuted transition table on device.

Host-side prep is integer-derived only: dtype-preserving copies
(packing, gather, transpose), constant fills, and integer counts. All
float arithmetic on pred/transitions happens on-device.
"""

import os
import sys

for _p in ("/opt/trn_rl_repo", "/root/.axon_site/_ro/trn_rl_repo"):
    if os.path.isdir(_p) and _p not in sys.path:
        sys.path.insert(0, _p)

import numpy as np

B, T, L = 1024, 512, 62
K = 64
NCORES = 8
BC = 128                     # sequences per core
H = 258                      # forward steps
NJB = 513 - H + 1            # backward steps: inject + (s-H) live + final
SMALL = -1000.0
R_RESCALE = 16
LAG = 4
CBIAS = float(np.float32(4.4))   # per-step drift cancel: e tiles scaled e^-C
CH = 8                       # steps per DMA/exp chunk (small => ACT queue
                             # stays fine-grained; rescale ops don't stall)
LOOKAHEAD = 3                # chunks issued ahead of the chain
LN2_32 = 32.0 * float(np.log(2.0))

# permutation: new index k' -> old label index (0=end, 1=start, 2+l=label l)
PERM = np.concatenate(([63, 62], np.arange(62)))

_PROGRAM_CACHE = {}


# --------------------------------------------------------------------------
# host-side packing (integer-derived only)
# --------------------------------------------------------------------------

def _events(n_steps, offset=0):
    """Staggered rescale decision steps; apply (at +LAG) must land."""
    return list(range(R_RESCALE + offset, n_steps - LAG - 1, R_RESCALE))


def _pack(pred, ref, seq_len):
    pred = np.ascontiguousarray(pred, dtype=np.float32)
    ref64 = np.asarray(ref).astype(np.int64)
    s_all = np.asarray(seq_len).astype(np.int64)

    order = np.argsort(-s_all, kind="stable")
    assign = np.empty((NCORES, BC), dtype=np.int64)
    assign[np.arange(B) % NCORES, np.arange(B) // NCORES] = order
    s = s_all[assign]                                   # (C, 128)
    nlong = (s >= H).sum(axis=1)
    W_b = max(1, int(np.ceil(nlong.max() / 2)))

    # ---- forward obs [C, H, 128, 64] -------------------------------------
    obs_f = np.empty((NCORES, H, 128, 64), np.float32)
    obs_f.fill(SMALL)
    jj = np.arange(H)
    for g in (0, 1):
        qs = np.arange(g, BC, 2)
        idx = assign[:, qs]                             # (C, 64)
        sg = s[:, qs]                                   # (C, 64)
        vals = pred[idx, :H, :]                         # (C, 64, H, 62)
        live = jj[None, :, None] < sg[:, None, :]       # (C, H, 64)
        obs_f[:, :, 64 * g + 2 : 64 * g + 64, :] = np.where(
            live[:, :, None, :], vals.transpose(0, 2, 3, 1), np.float32(SMALL)
        )
        # extraction row (k'=0): jj == s  ->  obs 0.0
        ext = jj[None, :, None] == sg[:, None, :]
        obs_f[:, :, 64 * g + 0, :] = np.where(ext, 0.0, SMALL).astype(np.float32)
        # death rows (k'=1): jj > s -> keep-alive +C (cancels the e^-C bias)
        dead = jj[None, :, None] > sg[:, None, :]
        obs_f[:, :, 64 * g + 1, :] = np.where(dead, CBIAS, SMALL).astype(np.float32)

    # ---- backward obs [C, NJB, 128, W_b] ---------------------------------
    obs_b = np.empty((NCORES, NJB, 128, W_b), np.float32)
    obs_b.fill(SMALL)
    rr = np.arange(NJB)                                  # rr = r-1
    for g in (0, 1):
        qs = np.arange(g, 2 * W_b, 2)
        idx = assign[:, qs]                              # (C, W_b)
        sg = s[:, qs]                                    # (C, W_b)
        long = sg >= H                                   # (C, W_b)
        r0 = 513 - sg                                    # valid where long
        r = rr[None, :, None] + 1                        # (1, NJB, 1)
        # emission rows: live where long & r0 < r < NJB; time = 513-r
        liv = long[:, None, :] & (r > r0[:, None, :]) & (r < NJB)
        vals = pred[idx, ::-1, :][:, :, : NJB - 1, :]    # (C, W_b, NJB-1, 62)
        blk = np.where(
            liv[:, : NJB - 1, None, :],
            vals.transpose(0, 2, 3, 1),
            np.float32(SMALL),
        )
        obs_b[:, : NJB - 1, 64 * g + 2 : 64 * g + 64, :] = blk
        # row 0 (end'): +C on hold (r < r0) or short-hold, 0.0 at inject
        hold = np.where(long[:, None, :], r < r0[:, None, :], True)
        inj = long[:, None, :] & (r == r0[:, None, :])
        row0v = np.where(hold, CBIAS, np.where(inj, 0.0, SMALL))
        obs_b[:, :, 64 * g + 0, :] = row0v.astype(np.float32)
        # row 1 (keep-alive): +C on hold only
        obs_b[:, :, 64 * g + 1, :] = np.where(hold, CBIAS, SMALL).astype(np.float32)
    obs_b[:, NJB - 1, :, :] = 0.0                        # final all-ones step

    # ---- gold emissions (host integer gather, masked) --------------------
    pred_d = pred[assign]                                # (C, 128, T, 62)
    gold = np.take_along_axis(pred_d, ref64[assign][..., None], axis=3)[..., 0]
    tmask = np.arange(T)[None, None, :] < s[:, :, None]
    gold = np.where(tmask, gold, np.float32(0.0)).astype(np.float32)

    # ---- gold transition pair counts (permuted space) --------------------
    cmat = np.zeros((NCORES, K, K), dtype=np.int64)
    ref_d = ref64[assign]                                # (C, 128, T)
    for c in range(NCORES):
        for q in range(BC):
            sq = int(s[c, q])
            path = np.concatenate(([1], ref_d[c, q, :sq] + 2, [0]))
            np.add.at(cmat[c], (path[:-1], path[1:]), 1)

    # boosted-step counts per core (for the e^-C bias correction)
    nb_f = np.maximum(0, H - s - 1).sum(axis=1)                  # (C,)
    sq_b = s[:, : 2 * W_b]
    nb_b = np.where(sq_b >= H, 512 - sq_b, NJB - 1).sum(axis=1)  # (C,)
    bcount = (nb_f + nb_b).astype(np.float32).reshape(NCORES, 1, 1)
    return obs_f, obs_b, gold, cmat.astype(np.float32), W_b, bcount


# --------------------------------------------------------------------------
# device program
# --------------------------------------------------------------------------

class _Chain:
    """One serial chain object: state tile + pools + rescale bookkeeping."""

    def __init__(self, name, w, c0, lhsT, arena, lnbuf, lcol0, apool, ppool):
        self.name = name
        self.w = w                  # free width
        self.c0 = c0                # slot offset inside arena/lnbuf
        self.lhsT = lhsT
        self.arena = arena
        self.lnbuf = lnbuf
        self.lcol0 = lcol0          # column offset inside lnbuf
        self.apool = apool
        self.ppool = ppool
        self.a = None
        self.pending = {}           # apply_step -> bcs sbuf tile


def _build_program(W_b):
    import concourse.bacc as bacc
    import concourse.tile as tile
    from concourse import mybir

    f32 = mybir.dt.float32
    bf16 = mybir.dt.bfloat16
    AF = mybir.ActivationFunctionType
    ALU = mybir.AluOpType
    AX = mybir.AxisListType

    ev_f0 = _events(H, 0)
    ev_f1 = _events(H, 5)
    ev_b = _events(NJB, 11)
    assert len(ev_f0) == len(ev_f1)
    nev_f, nev_b = len(ev_f0), len(ev_b)

    nc = bacc.Bacc()
    obsf_d = nc.dram_tensor("obsf", [H, 128, 64], f32, kind="ExternalInput")
    obsb_d = nc.dram_tensor("obsb", [NJB, 128, W_b], f32, kind="ExternalInput")
    gold_d = nc.dram_tensor("gold", [128, T], f32, kind="ExternalInput")
    trans_d = nc.dram_tensor("trans", [K, K], f32, kind="ExternalInput")
    transfp_d = nc.dram_tensor("transfp", [K, K], f32, kind="ExternalInput")
    transb_d = nc.dram_tensor("transb", [K, K], f32, kind="ExternalInput")
    bcw_d = nc.dram_tensor("bcwp", [2, 128], f32, kind="ExternalInput")
    bcount_d = nc.dram_tensor("bcount", [1, 1], f32, kind="ExternalInput")
    cmat_d = nc.dram_tensor("cmat", [K, K], f32, kind="ExternalInput")
    out_d = nc.dram_tensor("out", [1, 8], f32, kind="ExternalOutput")

    with tile.TileContext(nc) as tc:
        with (
            tc.tile_pool(name="const", bufs=1) as const,
            tc.tile_pool(name="stgf", bufs=3) as stgf,
            tc.tile_pool(name="stgb", bufs=3) as stgb,
            tc.tile_pool(name="aF0", bufs=3) as aF0p,
            tc.tile_pool(name="aF1", bufs=3) as aF1p,
            tc.tile_pool(name="aB0", bufs=3) as aB0p,
            tc.tile_pool(name="rsc", bufs=4) as rsc,
            tc.tile_pool(name="endp", bufs=1) as endp,
            tc.tile_pool(name="pF0", bufs=1, space="PSUM") as pF0,
            tc.tile_pool(name="pF1", bufs=1, space="PSUM") as pF1,
            tc.tile_pool(name="pB0", bufs=1, space="PSUM") as pB0,
            tc.tile_pool(name="pmisc", bufs=1, space="PSUM") as pmisc,
        ):
            # ---- constants -----------------------------------------------
            trans_s = const.tile([K, K], f32)
            nc.gpsimd.dma_start(out=trans_s, in_=trans_d[:, :])
            transfp_s = const.tile([K, K], f32)
            nc.gpsimd.dma_start(out=transfp_s, in_=transfp_d[:, :])
            transb_s = const.tile([K, K], f32)
            nc.gpsimd.dma_start(out=transb_s, in_=transb_d[:, :])
            cmat_s = const.tile([K, K], f32)
            nc.gpsimd.dma_start(out=cmat_s, in_=cmat_d[:, :])
            bcw_f32 = const.tile([2, 128], f32)
            nc.gpsimd.dma_start(out=bcw_f32, in_=bcw_d[:, :])
            bcount_s = const.tile([1, 1], f32)
            nc.gpsimd.dma_start(out=bcount_s, in_=bcount_d[:, :])

            # stationaries: block-diag exp() of the host-plumbed matrices
            E2f = const.tile([128, 128], bf16)
            nc.vector.memset(E2f, 0.0)
            M2b = const.tile([128, 128], bf16)
            nc.vector.memset(M2b, 0.0)
            for g in (0, 1):
                o = 64 * g
                nc.scalar.activation(
                    out=E2f[o : o + K, o : o + K], in_=transfp_s, func=AF.Exp
                )
                nc.scalar.activation(
                    out=M2b[o : o + K, o : o + K], in_=transb_s, func=AF.Exp
                )

            # per-group colsum selector [128, 2] and broadcast weights [2, 128]
            sel2 = const.tile([128, 2], bf16)
            nc.vector.memset(sel2, 0.0)
            nc.vector.memset(sel2[0:64, 0:1], 1.0)
            nc.vector.memset(sel2[64:128, 1:2], 1.0)
            sel2f = const.tile([128, 2], f32)
            nc.vector.memset(sel2f, 0.0)
            nc.vector.memset(sel2f[0:64, 0:1], 1.0)
            nc.vector.memset(sel2f[64:128, 1:2], 1.0)
            bcw = const.tile([2, 128], bf16)
            nc.scalar.copy(out=bcw, in_=bcw_f32)
            ones_col = const.tile([128, 1], f32)
            nc.vector.memset(ones_col, 1.0)
            ebias = const.tile([128, 1], f32)       # exp bias: -C per step
            nc.vector.memset(ebias, -CBIAS)

            # chain initial states
            a0f = const.tile([128, 64], bf16)
            nc.vector.memset(a0f, 1.0)
            nc.vector.memset(a0f[0:2, :], 0.0)
            nc.vector.memset(a0f[64:66, :], 0.0)
            b0b = const.tile([128, W_b], bf16)
            nc.vector.memset(b0b, 0.0)
            nc.vector.memset(b0b[0:2, :], 1.0)
            nc.vector.memset(b0b[64:66, :], 1.0)

            # raw per-event colsum stash (ln'd in one batched op at endgame)
            csbuf_f = const.tile([2, nev_f + 1, 64], f32)
            csbuf_b = const.tile([2, nev_b + 1, W_b], f32)

            # ---- gold terms (early; engines otherwise idle) --------------
            gold_s = const.tile([128, T], f32)
            nc.sync.dma_start(out=gold_s, in_=gold_d[:, :])
            gacc = const.tile([128, 1], f32)
            nc.scalar.activation(
                out=gold_s, in_=gold_s, func=AF.Copy, accum_out=gacc
            )
            trans_st = const.tile([K, K], f32)
            nc.scalar.copy(out=trans_st, in_=trans_s)
            cmat_st = const.tile([K, K], f32)
            nc.scalar.copy(out=cmat_st, in_=cmat_s)
            scr = const.tile([K, K], f32)
            nc.vector.tensor_mul(scr, trans_st, cmat_st)
            gt = const.tile([K, 1], f32)
            nc.vector.tensor_reduce(out=gt, in_=scr, axis=AX.X, op=ALU.add)

            # ---- exp arenas (bf16), streamed in CH-step chunks -----------
            e_f = const.tile([128, H, 64], bf16)
            e_b = const.tile([128, NJB, W_b], bf16)
            nchunk_f = (H + CH - 1) // CH
            nchunk_b = (NJB + CH - 1) // CH

            def issue_chunk_f(ci):
                j0 = ci * CH
                cw = min(CH, H - j0)
                st = stgf.tile([128, CH, 64], f32, tag="stf")
                nc.sync.dma_start(
                    out=st[:, :cw, :],
                    in_=obsf_d[j0 : j0 + cw].rearrange("j p b -> p j b"),
                )
                nc.scalar.activation(
                    out=e_f[:, j0 : j0 + cw, :], in_=st[:, :cw, :], func=AF.Exp,
                    bias=ebias,
                )

            def issue_chunk_b(ci):
                j0 = ci * CH
                cw = min(CH, NJB - j0)
                st = stgb.tile([128, CH, W_b], f32, tag="stb")
                nc.sync.dma_start(
                    out=st[:, :cw, :],
                    in_=obsb_d[j0 : j0 + cw].rearrange("j p b -> p j b"),
                )
                nc.scalar.activation(
                    out=e_b[:, j0 : j0 + cw, :], in_=st[:, :cw, :], func=AF.Exp,
                    bias=ebias,
                )

            # ---- chain objects -------------------------------------------
            F0 = _Chain("F0", 32, 0, E2f, e_f, csbuf_f, 0, aF0p, pF0)
            F1 = _Chain("F1", 32, 32, E2f, e_f, csbuf_f, 32, aF1p, pF1)
            B0 = _Chain("B0", W_b, 0, M2b, e_b, csbuf_b, 0, aB0p, pB0)
            F0.a, F1.a = a0f[:, 0:32], a0f[:, 32:64]
            B0.a = b0b
            ev_sets = {"F0": (set(ev_f0), ev_f0), "F1": (set(ev_f1), ev_f1),
                       "B0": (set(ev_b), ev_b)}

            def chain_step(obj, j, ev_set, ev_list):
                # pending rescale apply: premultiply the arena slice (GPSIMD,
                # SBUF-only) BEFORE this step's mul reads it
                bcs = obj.pending.pop(j, None)
                esl = obj.arena[:, j - 1, obj.c0 : obj.c0 + obj.w]
                if bcs is not None:
                    nc.gpsimd.tensor_mul(esl, esl, bcs)
                ps = obj.ppool.tile([128, obj.w], f32, tag=f"ps{obj.name}")
                nc.tensor.matmul(
                    ps, lhsT=obj.lhsT, rhs=obj.a, start=True, stop=True
                )
                anew = obj.apool.tile([128, obj.w], bf16, tag=f"a{obj.name}")
                nc.vector.tensor_mul(anew, ps, esl)
                obj.a = anew
                if j in ev_set:
                    ri = ev_list.index(j)
                    cs_full = pmisc.tile([2, 64], f32, tag="cs")
                    cs = cs_full[:, : obj.w]
                    nc.tensor.matmul(
                        cs, lhsT=sel2, rhs=anew, start=True, stop=True
                    )
                    csl = obj.lnbuf[:, ri, obj.lcol0 : obj.lcol0 + obj.w]
                    nc.scalar.activation(out=csl, in_=cs, func=AF.Copy)
                    bcx = rsc.tile([2, obj.w], bf16, tag="bcx")
                    with nc.allow_low_precision(reason="rescale factor; drift lands in tracked shifts"):
                        nc.vector.reciprocal(bcx, cs)
                    psb_full = pmisc.tile([128, 64], f32, tag="psb")
                    psb = psb_full[:, : obj.w]
                    nc.tensor.matmul(
                        psb, lhsT=bcw, rhs=bcx, start=True, stop=True
                    )
                    bcs2 = rsc.tile([128, obj.w], bf16, tag="bcs")
                    nc.scalar.activation(out=bcs2, in_=psb, func=AF.Copy)
                    obj.pending[j + LAG] = bcs2

            # prologue chunks
            nf = nb = 0
            for _ in range(LOOKAHEAD):
                if nf < nchunk_f:
                    issue_chunk_f(nf)
                    nf += 1
                if nb < nchunk_b:
                    issue_chunk_b(nb)
                    nb += 1

            for it in range(1, H + 1):
                while nf < nchunk_f and nf * CH < it + LOOKAHEAD * CH:
                    issue_chunk_f(nf)
                    nf += 1
                while nb < nchunk_b and nb * CH < it + LOOKAHEAD * CH:
                    issue_chunk_b(nb)
                    nb += 1
                chain_step(F0, it, *ev_sets["F0"])
                chain_step(F1, it, *ev_sets["F1"])
                if it <= NJB:
                    chain_step(B0, it, *ev_sets["B0"])

            # ---- endgame --------------------------------------------------
            def final_norm(obj, nev):
                cs_full = pmisc.tile([2, 64], f32, tag="cs")
                cs = cs_full[:, : obj.w]
                nc.tensor.matmul(cs, lhsT=sel2, rhs=obj.a, start=True, stop=True)
                csl = obj.lnbuf[:, nev, obj.lcol0 : obj.lcol0 + obj.w]
                nc.scalar.activation(out=csl, in_=cs, func=AF.Copy)
                bcx = rsc.tile([2, obj.w], bf16, tag="bcx")
                with nc.allow_low_precision(reason="rescale factor; drift lands in tracked shifts"):
                    nc.vector.reciprocal(bcx, cs)
                psb_full = pmisc.tile([128, 64], f32, tag="psb")
                psb = psb_full[:, : obj.w]
                nc.tensor.matmul(psb, lhsT=bcw, rhs=bcx, start=True, stop=True)
                an = endp.tile([128, obj.w], f32, tag=f"an{obj.name}")
                nc.vector.tensor_mul(an, psb, obj.a)
                return an

            anF0 = final_norm(F0, nev_f)
            anF1 = final_norm(F1, nev_f)
            anB0 = final_norm(B0, nev_b)

            gm = endp.tile([128, 64], f32)
            nc.vector.memset(gm, 0.0)
            nc.vector.memset(gm[0:2, :], 1.0)
            nc.vector.memset(gm[64:66, :], 1.0)
            nc.vector.tensor_copy(out=gm[:, 0:W_b], in_=anB0)

            m0 = endp.tile([128, 64], f32)
            nc.vector.tensor_mul(m0[:, 0:32], anF0, gm[:, 0:32])
            nc.vector.tensor_mul(m0[:, 32:64], anF1, gm[:, 32:64])
            psd = pmisc.tile([2, 64], f32, tag="cs")
            nc.tensor.matmul(psd, lhsT=sel2f, rhs=m0, start=True, stop=True)
            lnd = endp.tile([2, 64], f32)
            nc.scalar.activation(out=lnd, in_=psd, func=AF.Ln)

            # 2^-32 prescale keeps the Ln arg inside ScalarE's +-2^64 domain;
            # the +32ln2 per event is folded into the final bias constant
            lnbuf_f = endp.tile([2, nev_f + 1, 64], f32)
            nc.scalar.activation(
                out=lnbuf_f, in_=csbuf_f, func=AF.Ln, scale=float(2.0 ** -32)
            )
            lnbuf_b = endp.tile([2, nev_b + 1, W_b], f32)
            nc.scalar.activation(
                out=lnbuf_b, in_=csbuf_b, func=AF.Ln, scale=float(2.0 ** -32)
            )
            ssf = endp.tile([2, 64], f32)
            nc.vector.tensor_reduce(
                out=ssf, in_=lnbuf_f.rearrange("p a b -> p b a"),
                axis=AX.X, op=ALU.add,
            )
            ssb = endp.tile([2, W_b], f32)
            nc.vector.tensor_reduce(
                out=ssb, in_=lnbuf_b.rearrange("p a b -> p b a"),
                axis=AX.X, op=ALU.add,
            )
            tot = endp.tile([2, 64], f32)
            nc.vector.tensor_add(tot, lnd, ssf)
            nc.vector.tensor_add(tot[:, 0:W_b], tot[:, 0:W_b], ssb)
            red = endp.tile([2, 1], f32)
            nc.vector.tensor_reduce(out=red, in_=tot, axis=AX.X, op=ALU.add)
            ones2 = endp.tile([2, 1], f32)
            nc.vector.memset(ones2, 1.0)
            zps = pmisc.tile([1, 1], f32, tag="sc1")
            nc.tensor.matmul(zps, lhsT=red, rhs=ones2, start=True, stop=True)
            bias = (
                BC * (LN2_32 * (nev_f + 1) - 1000.0 + CBIAS * H)
                + 2.0 * W_b * (LN2_32 * (nev_b + 1) + CBIAS * NJB)
            )
            szl = endp.tile([1, 1], f32)
            nc.scalar.activation(
                out=szl, in_=zps, func=AF.Copy, bias=float(bias), scale=1.0
            )
            bcc = endp.tile([1, 1], f32)
            nc.scalar.activation(
                out=bcc, in_=bcount_s, func=AF.Copy, bias=0.0, scale=float(CBIAS)
            )
            nc.vector.tensor_sub(szl, szl, bcc)

            ge_ps = pmisc.tile([1, 1], f32, tag="sc1")
            nc.tensor.matmul(
                ge_ps, lhsT=gacc, rhs=ones_col, start=True, stop=True
            )
            gesb = endp.tile([1, 1], f32)
            nc.vector.tensor_copy(out=gesb, in_=ge_ps)
            gt_ps = pmisc.tile([1, 1], f32, tag="sc1")
            nc.tensor.matmul(
                gt_ps, lhsT=gt, rhs=ones_col[0:K, :], start=True, stop=True
            )

            fin = endp.tile([1, 8], f32)
            nc.vector.tensor_sub(fin[:, 0:1], szl, gesb)
            nc.vector.tensor_sub(fin[:, 0:1], fin[:, 0:1], gt_ps)
            nc.vector.tensor_copy(out=fin[:, 1:2], in_=szl)
            nc.vector.tensor_copy(out=fin[:, 2:3], in_=gesb)
            nc.vector.tensor_copy(out=fin[:, 3:4], in_=gt_ps)
            nc.vector.memset(fin[:, 4:8], 0.0)
            nc.sync.dma_start(out=out_d[:, :], in_=fin)

    nc.compile()
    return nc


def _get_program(W_b):
    if W_b not in _PROGRAM_CACHE:
        _PROGRAM_CACHE[W_b] = _build_program(W_b)
    return _PROGRAM_CACHE[W_b]


# --------------------------------------------------------------------------
# entry point
# --------------------------------------------------------------------------

def kernel(pred, ref, seq_len, transitions):
    from concourse.bass_utils import run_bass_kernel_spmd

    obs_f, obs_b, gold, cmat, W_b, bcount = _pack(pred, ref, seq_len)
    trans_f = np.ascontiguousarray(
        np.asarray(transitions, dtype=np.float32)[np.ix_(PERM, PERM)]
    )
    # plumbing as constant fills (mirrors the reference's own -10000 fills):
    # fwd: rows 0/1 (from end/keep-alive) -> 0.0 so exp()=1 keep-alive rows
    transf_p = trans_f.copy()
    transf_p[0, :] = 0.0
    transf_p[1, :] = 0.0
    # bwd stationary (lhsT = E^T with hold/inject plumbing):
    #   row1 = e1 (hold), [0,0]=1, [0,1]=0; rows>=2 cols 0/1 already -10000
    transb_p = np.ascontiguousarray(trans_f.T)
    transb_p[1, :] = -10000.0
    transb_p[1, 1] = 0.0
    transb_p[0, 0] = 0.0
    transb_p[0, 1] = -10000.0
    # partition-broadcast stationary pattern [2, 128]
    bcw_np = np.zeros((2, 128), np.float32)
    bcw_np[0, 0:64] = 1.0
    bcw_np[1, 64:128] = 1.0

    nc = _get_program(W_b)
    in_maps = [
        {
            "obsf": np.ascontiguousarray(obs_f[c]),
            "obsb": np.ascontiguousarray(obs_b[c]),
            "gold": np.ascontiguousarray(gold[c]),
            "trans": trans_f,
            "transfp": transf_p,
            "transb": transb_p,
            "bcwp": bcw_np,
            "cmat": np.ascontiguousarray(cmat[c]),
            "bcount": np.ascontiguousarray(bcount[c]),
        }
        for c in range(NCORES)
    ]
    total = np.float64(np.nan)
    for _attempt in range(3):
        res = run_bass_kernel_spmd(
            nc, in_maps, list(range(NCORES)),
            trace=bool(os.environ.get("BASS_TRACE")),
        )
        if res.exec_time_ns is not None:
            print(f"HW exec time: {res.exec_time_ns} ns")
        total = np.float64(0.0)
        for c in range(NCORES):
            total += np.float64(res.results[c]["out"][0, 0])
        if np.isfinite(total):
            break
    return np.array(np.float32(total))


# revision 16
# speedup vs baseline: 3.3097x; 1.0895x over previous
"""CRF loss (nn_CRFLossOld) on 8 Trainium2 NeuronCores.

Forward/backward-split linear-domain CRF:

  - Sequences are sorted by length (desc) and dealt round-robin to the 8
    cores; per core, rank q -> (group g=q%2, slot c=q//2) so both
    partition-halves of each tile carry 64 columns.
  - FORWARD chain (H=258 steps): a_j = (E2f^T a_{j-1}) * exp(obs_j) with
    E2f a 128x128 block-diagonal bf16 stationary (two copies of the
    64x64 exp'd transition matrix; rows 0/1 of each block are the
    keep-alive plumbing, =1). Short sequences (s+1 <= H) complete here
    (extraction at s+1, then keep-alive holds w in rows 0/1).
  - BACKWARD chain (NJB=256 steps) for long sequences, run on the
    e-shifted suffix state bt_j = e_j . beta_j, time-reversed and
    delayed so ALL columns finish at step NJB with beta_H:
      r < r0=513-s: hold [1,1,0..]   r==r0: inject bt_{s+1}=[1,0,..]
      r0<r<NJB: live (global time 513-r)   r==NJB: all-ones e => beta_H
    Stationary M2b = block-diag exp(trans^T) with plumbing rows/cols
    (see _build_program).
  - MERGE: per column Z = sum_k alpha_H[k]*gamma[k]; gamma = beta_H for
    long slots (first W_b free slots), [1,1,0..] for the rest. The dot
    telescopes both chains' per-column rescale shifts.
  - Rescales every R=8 steps per chain: per-group colsum via a tiny
    [128,2] selector matmul, ln on ACT (2^-32 prescale, feeding the
    shift accumulator), 1/cs via ACT exp(-x-32ln2), partition-broadcast
    via a [2,128] matmul, applied LAG=4 steps later by premultiplying
    the exp(obs) arena slice on GPSIMD (SBUF-only operands).
  - Chain arithmetic is bf16 (PE 1 cycle/row; matmuls accumulate fp32
    in PSUM), exact bookkeeping in fp32 shifts.
  - Gold path score: emissions are host-gathered (dtype-preserving
    integer-indexed copy of pred) and summed with one ACT accumulate;
    transitions via per-core integer pair-count matrix dotted with the
    permuted transition table on device.

Host-side prep is integer-derived only: dtype-preserving copies
(packing, gather, transpose), constant fills, and integer counts. All
float arithmetic on pred/transitions happens on-device.
"""

import os
import sys

for _p in ("/opt/trn_rl_repo", "/root/.axon_site/_ro/trn_rl_repo"):
    if os.path.isdir(_p) and _p not in sys.path:
        sys.path.insert(0, _p)

import numpy as np

B, T, L = 1024, 512, 62
K = 64
NCORES = 8
BC = 128                     # sequences per core
H = 258                      # forward steps
NJB = 513 - H + 1            # backward steps: inject + (s-H) live + final
SMALL = -1000.0
R_RESCALE = 32
LAG = 4
CBIAS = float(np.float32(4.4))   # per-step drift cancel: e tiles scaled e^-C
CH = 8                       # steps per DMA/exp chunk (small => ACT queue
                             # stays fine-grained; rescale ops don't stall)
LOOKAHEAD = 3                # chunks issued ahead of the chain
LN2_32 = 32.0 * float(np.log(2.0))

# permutation: new index k' -> old label index (0=end, 1=start, 2+l=label l)
PERM = np.concatenate(([63, 62], np.arange(62)))

_PROGRAM_CACHE = {}


# --------------------------------------------------------------------------
# host-side packing (integer-derived only)
# --------------------------------------------------------------------------

def _events(n_steps, offset=0):
    """Staggered rescale decision steps; apply (at +LAG) must land."""
    return list(range(R_RESCALE + offset, n_steps - LAG - 1, R_RESCALE))


def _pack(pred, ref, seq_len):
    pred = np.ascontiguousarray(pred, dtype=np.float32)
    ref64 = np.asarray(ref).astype(np.int64)
    s_all = np.asarray(seq_len).astype(np.int64)

    order = np.argsort(-s_all, kind="stable")
    assign = np.empty((NCORES, BC), dtype=np.int64)
    assign[np.arange(B) % NCORES, np.arange(B) // NCORES] = order
    s = s_all[assign]                                   # (C, 128)
    nlong = (s >= H).sum(axis=1)
    W_b = max(1, int(np.ceil(nlong.max() / 2)))

    # ---- forward obs [C, H, 128, 64] -------------------------------------
    obs_f = np.empty((NCORES, H, 128, 64), np.float32)
    obs_f.fill(SMALL)
    jj = np.arange(H)
    for g in (0, 1):
        qs = np.arange(g, BC, 2)
        idx = assign[:, qs]                             # (C, 64)
        sg = s[:, qs]                                   # (C, 64)
        vals = pred[idx, :H, :]                         # (C, 64, H, 62)
        live = jj[None, :, None] < sg[:, None, :]       # (C, H, 64)
        obs_f[:, :, 64 * g + 2 : 64 * g + 64, :] = np.where(
            live[:, :, None, :], vals.transpose(0, 2, 3, 1), np.float32(SMALL)
        )
        # extraction row (k'=0): jj == s  ->  obs 0.0
        ext = jj[None, :, None] == sg[:, None, :]
        obs_f[:, :, 64 * g + 0, :] = np.where(ext, 0.0, SMALL).astype(np.float32)
        # death rows (k'=1): jj > s -> keep-alive +C (cancels the e^-C bias)
        dead = jj[None, :, None] > sg[:, None, :]
        obs_f[:, :, 64 * g + 1, :] = np.where(dead, CBIAS, SMALL).astype(np.float32)

    # ---- backward obs [C, NJB, 128, W_b] ---------------------------------
    obs_b = np.empty((NCORES, NJB, 128, W_b), np.float32)
    obs_b.fill(SMALL)
    rr = np.arange(NJB)                                  # rr = r-1
    for g in (0, 1):
        qs = np.arange(g, 2 * W_b, 2)
        idx = assign[:, qs]                              # (C, W_b)
        sg = s[:, qs]                                    # (C, W_b)
        long = sg >= H                                   # (C, W_b)
        r0 = 513 - sg                                    # valid where long
        r = rr[None, :, None] + 1                        # (1, NJB, 1)
        # emission rows: live where long & r0 < r < NJB; time = 513-r
        liv = long[:, None, :] & (r > r0[:, None, :]) & (r < NJB)
        vals = pred[idx, ::-1, :][:, :, : NJB - 1, :]    # (C, W_b, NJB-1, 62)
        blk = np.where(
            liv[:, : NJB - 1, None, :],
            vals.transpose(0, 2, 3, 1),
            np.float32(SMALL),
        )
        obs_b[:, : NJB - 1, 64 * g + 2 : 64 * g + 64, :] = blk
        # row 0 (end'): +C on hold (r < r0) or short-hold, 0.0 at inject
        hold = np.where(long[:, None, :], r < r0[:, None, :], True)
        inj = long[:, None, :] & (r == r0[:, None, :])
        row0v = np.where(hold, CBIAS, np.where(inj, 0.0, SMALL))
        obs_b[:, :, 64 * g + 0, :] = row0v.astype(np.float32)
        # row 1 (keep-alive): +C on hold only
        obs_b[:, :, 64 * g + 1, :] = np.where(hold, CBIAS, SMALL).astype(np.float32)
    obs_b[:, NJB - 1, :, :] = 0.0                        # final all-ones step

    # ---- gold emissions (host integer gather, masked) --------------------
    pred_d = pred[assign]                                # (C, 128, T, 62)
    gold = np.take_along_axis(pred_d, ref64[assign][..., None], axis=3)[..., 0]
    tmask = np.arange(T)[None, None, :] < s[:, :, None]
    gold = np.where(tmask, gold, np.float32(0.0)).astype(np.float32)

    # ---- gold transition pair counts (permuted space) --------------------
    cmat = np.zeros((NCORES, K, K), dtype=np.int64)
    ref_d = ref64[assign]                                # (C, 128, T)
    for c in range(NCORES):
        for q in range(BC):
            sq = int(s[c, q])
            path = np.concatenate(([1], ref_d[c, q, :sq] + 2, [0]))
            np.add.at(cmat[c], (path[:-1], path[1:]), 1)

    # boosted-step counts per core (for the e^-C bias correction)
    nb_f = np.maximum(0, H - s - 1).sum(axis=1)                  # (C,)
    sq_b = s[:, : 2 * W_b]
    nb_b = np.where(sq_b >= H, 512 - sq_b, NJB - 1).sum(axis=1)  # (C,)
    bcount = (nb_f + nb_b).astype(np.float32).reshape(NCORES, 1, 1)
    return obs_f, obs_b, gold, cmat.astype(np.float32), W_b, bcount


# --------------------------------------------------------------------------
# device program
# --------------------------------------------------------------------------

class _Chain:
    """One serial chain object: state tile + pools + rescale bookkeeping."""

    def __init__(self, name, w, c0, lhsT, arena, lnbuf, lcol0, apool, ppool):
        self.name = name
        self.w = w                  # free width
        self.c0 = c0                # slot offset inside arena/lnbuf
        self.lhsT = lhsT
        self.arena = arena
        self.lnbuf = lnbuf
        self.lcol0 = lcol0          # column offset inside lnbuf
        self.apool = apool
        self.ppool = ppool
        self.a = None
        self.pending = {}           # apply_step -> bcs sbuf tile


def _build_program(W_b):
    import concourse.bacc as bacc
    import concourse.tile as tile
    from concourse import mybir

    f32 = mybir.dt.float32
    bf16 = mybir.dt.bfloat16
    AF = mybir.ActivationFunctionType
    ALU = mybir.AluOpType
    AX = mybir.AxisListType

    ev_f0 = _events(H, 0)
    ev_f1 = _events(H, 10)
    ev_b = _events(NJB, 21)
    assert len(ev_f0) == len(ev_f1)
    nev_f, nev_b = len(ev_f0), len(ev_b)

    nc = bacc.Bacc()
    obsf_d = nc.dram_tensor("obsf", [H, 128, 64], f32, kind="ExternalInput")
    obsb_d = nc.dram_tensor("obsb", [NJB, 128, W_b], f32, kind="ExternalInput")
    gold_d = nc.dram_tensor("gold", [128, T], f32, kind="ExternalInput")
    trans_d = nc.dram_tensor("trans", [K, K], f32, kind="ExternalInput")
    transfp_d = nc.dram_tensor("transfp", [K, K], f32, kind="ExternalInput")
    transb_d = nc.dram_tensor("transb", [K, K], f32, kind="ExternalInput")
    bcw_d = nc.dram_tensor("bcwp", [2, 128], f32, kind="ExternalInput")
    bcount_d = nc.dram_tensor("bcount", [1, 1], f32, kind="ExternalInput")
    cmat_d = nc.dram_tensor("cmat", [K, K], f32, kind="ExternalInput")
    out_d = nc.dram_tensor("out", [1, 8], f32, kind="ExternalOutput")

    with tile.TileContext(nc) as tc:
        with (
            tc.tile_pool(name="const", bufs=1) as const,
            tc.tile_pool(name="stgf", bufs=3) as stgf,
            tc.tile_pool(name="stgb", bufs=3) as stgb,
            tc.tile_pool(name="aF0", bufs=3) as aF0p,
            tc.tile_pool(name="aF1", bufs=3) as aF1p,
            tc.tile_pool(name="aB0", bufs=3) as aB0p,
            tc.tile_pool(name="rsc", bufs=4) as rsc,
            tc.tile_pool(name="endp", bufs=1) as endp,
            tc.tile_pool(name="pF0", bufs=1, space="PSUM") as pF0,
            tc.tile_pool(name="pF1", bufs=1, space="PSUM") as pF1,
            tc.tile_pool(name="pB0", bufs=1, space="PSUM") as pB0,
            tc.tile_pool(name="pmisc", bufs=1, space="PSUM") as pmisc,
        ):
            # ---- constants -----------------------------------------------
            trans_s = const.tile([K, K], f32)
            nc.gpsimd.dma_start(out=trans_s, in_=trans_d[:, :])
            transfp_s = const.tile([K, K], f32)
            nc.gpsimd.dma_start(out=transfp_s, in_=transfp_d[:, :])
            transb_s = const.tile([K, K], f32)
            nc.gpsimd.dma_start(out=transb_s, in_=transb_d[:, :])
            cmat_s = const.tile([K, K], f32)
            nc.gpsimd.dma_start(out=cmat_s, in_=cmat_d[:, :])
            bcw_f32 = const.tile([2, 128], f32)
            nc.gpsimd.dma_start(out=bcw_f32, in_=bcw_d[:, :])
            bcount_s = const.tile([1, 1], f32)
            nc.gpsimd.dma_start(out=bcount_s, in_=bcount_d[:, :])

            # stationaries: block-diag exp() of the host-plumbed matrices
            E2f = const.tile([128, 128], bf16)
            nc.vector.memset(E2f, 0.0)
            M2b = const.tile([128, 128], bf16)
            nc.vector.memset(M2b, 0.0)
            for g in (0, 1):
                o = 64 * g
                nc.scalar.activation(
                    out=E2f[o : o + K, o : o + K], in_=transfp_s, func=AF.Exp
                )
                nc.scalar.activation(
                    out=M2b[o : o + K, o : o + K], in_=transb_s, func=AF.Exp
                )

            # per-group colsum selector [128, 2] and broadcast weights [2, 128]
            sel2 = const.tile([128, 2], bf16)
            nc.vector.memset(sel2, 0.0)
            nc.vector.memset(sel2[0:64, 0:1], 1.0)
            nc.vector.memset(sel2[64:128, 1:2], 1.0)
            sel2f = const.tile([128, 2], f32)
            nc.vector.memset(sel2f, 0.0)
            nc.vector.memset(sel2f[0:64, 0:1], 1.0)
            nc.vector.memset(sel2f[64:128, 1:2], 1.0)
            bcw = const.tile([2, 128], bf16)
            nc.scalar.copy(out=bcw, in_=bcw_f32)
            ones_col = const.tile([128, 1], f32)
            nc.vector.memset(ones_col, 1.0)
            ebias = const.tile([128, 1], f32)       # exp bias: -C per step
            nc.vector.memset(ebias, -CBIAS)

            # chain initial states
            a0f = const.tile([128, 64], bf16)
            nc.vector.memset(a0f, 1.0)
            nc.vector.memset(a0f[0:2, :], 0.0)
            nc.vector.memset(a0f[64:66, :], 0.0)
            b0b = const.tile([128, W_b], bf16)
            nc.vector.memset(b0b, 0.0)
            nc.vector.memset(b0b[0:2, :], 1.0)
            nc.vector.memset(b0b[64:66, :], 1.0)

            # raw per-event colsum stash (ln'd in one batched op at endgame)
            csbuf_f = const.tile([2, nev_f + 1, 64], f32)
            csbuf_b = const.tile([2, nev_b + 1, W_b], f32)

            # ---- exp arenas (bf16), streamed in CH-step chunks -----------
            e_f = const.tile([128, H, 64], bf16)
            e_b = const.tile([128, NJB, W_b], bf16)
            nchunk_f = (H + CH - 1) // CH
            nchunk_b = (NJB + CH - 1) // CH

            def issue_chunk_f(ci):
                j0 = ci * CH
                cw = min(CH, H - j0)
                st = stgf.tile([128, CH, 64], f32, tag="stf")
                nc.sync.dma_start(
                    out=st[:, :cw, :],
                    in_=obsf_d[j0 : j0 + cw].rearrange("j p b -> p j b"),
                )
                nc.scalar.activation(
                    out=e_f[:, j0 : j0 + cw, :], in_=st[:, :cw, :], func=AF.Exp,
                    bias=ebias,
                )

            def issue_chunk_b(ci):
                j0 = ci * CH
                cw = min(CH, NJB - j0)
                st = stgb.tile([128, CH, W_b], f32, tag="stb")
                nc.sync.dma_start(
                    out=st[:, :cw, :],
                    in_=obsb_d[j0 : j0 + cw].rearrange("j p b -> p j b"),
                )
                nc.scalar.activation(
                    out=e_b[:, j0 : j0 + cw, :], in_=st[:, :cw, :], func=AF.Exp,
                    bias=ebias,
                )

            # ---- chain objects -------------------------------------------
            F0 = _Chain("F0", 32, 0, E2f, e_f, csbuf_f, 0, aF0p, pF0)
            F1 = _Chain("F1", 32, 32, E2f, e_f, csbuf_f, 32, aF1p, pF1)
            B0 = _Chain("B0", W_b, 0, M2b, e_b, csbuf_b, 0, aB0p, pB0)
            F0.a, F1.a = a0f[:, 0:32], a0f[:, 32:64]
            B0.a = b0b
            ev_sets = {"F0": (set(ev_f0), ev_f0), "F1": (set(ev_f1), ev_f1),
                       "B0": (set(ev_b), ev_b)}

            def chain_step(obj, j, ev_set, ev_list):
                # pending rescale apply: premultiply the arena slice (GPSIMD,
                # SBUF-only) BEFORE this step's mul reads it
                bcs = obj.pending.pop(j, None)
                esl = obj.arena[:, j - 1, obj.c0 : obj.c0 + obj.w]
                if bcs is not None:
                    nc.gpsimd.tensor_mul(esl, esl, bcs)
                ps = obj.ppool.tile([128, obj.w], f32, tag=f"ps{obj.name}")
                nc.tensor.matmul(
                    ps, lhsT=obj.lhsT, rhs=obj.a, start=True, stop=True
                )
                anew = obj.apool.tile([128, obj.w], bf16, tag=f"a{obj.name}")
                nc.vector.tensor_mul(anew, ps, esl)
                obj.a = anew
                if j in ev_set:
                    ri = ev_list.index(j)
                    cs_full = pmisc.tile([2, 64], f32, tag="cs")
                    cs = cs_full[:, : obj.w]
                    nc.tensor.matmul(
                        cs, lhsT=sel2, rhs=anew, start=True, stop=True
                    )
                    csl = obj.lnbuf[:, ri, obj.lcol0 : obj.lcol0 + obj.w]
                    nc.scalar.activation(out=csl, in_=cs, func=AF.Copy)
                    bcx = rsc.tile([2, obj.w], bf16, tag="bcx")
                    with nc.allow_low_precision(reason="rescale factor; drift lands in tracked shifts"):
                        nc.vector.reciprocal(bcx, cs)
                    psb_full = pmisc.tile([128, 64], f32, tag="psb")
                    psb = psb_full[:, : obj.w]
                    nc.tensor.matmul(
                        psb, lhsT=bcw, rhs=bcx, start=True, stop=True
                    )
                    bcs2 = rsc.tile([128, obj.w], bf16, tag="bcs")
                    nc.scalar.activation(out=bcs2, in_=psb, func=AF.Copy)
                    obj.pending[j + LAG] = bcs2

            # prologue chunks
            nf = nb = 0
            for _ in range(LOOKAHEAD):
                if nf < nchunk_f:
                    issue_chunk_f(nf)
                    nf += 1
                if nb < nchunk_b:
                    issue_chunk_b(nb)
                    nb += 1

            for it in range(1, H + 1):
                while nf < nchunk_f and nf * CH < it + LOOKAHEAD * CH:
                    issue_chunk_f(nf)
                    nf += 1
                while nb < nchunk_b and nb * CH < it + LOOKAHEAD * CH:
                    issue_chunk_b(nb)
                    nb += 1
                chain_step(F0, it, *ev_sets["F0"])
                chain_step(F1, it, *ev_sets["F1"])
                if it <= NJB:
                    chain_step(B0, it, *ev_sets["B0"])

            # ---- gold terms (issued after the chains; off their queues) --
            gold_s = const.tile([128, T], f32)
            nc.sync.dma_start(out=gold_s, in_=gold_d[:, :])
            gacc = const.tile([128, 1], f32)
            nc.scalar.activation(
                out=gold_s, in_=gold_s, func=AF.Copy, accum_out=gacc
            )
            trans_st = const.tile([K, K], f32)
            nc.scalar.copy(out=trans_st, in_=trans_s)
            cmat_st = const.tile([K, K], f32)
            nc.scalar.copy(out=cmat_st, in_=cmat_s)
            scr = const.tile([K, K], f32)
            nc.vector.tensor_mul(scr, trans_st, cmat_st)
            gt = const.tile([K, 1], f32)
            nc.vector.tensor_reduce(out=gt, in_=scr, axis=AX.X, op=ALU.add)

            # ---- endgame --------------------------------------------------
            def final_norm(obj, nev):
                cs_full = pmisc.tile([2, 64], f32, tag="cs")
                cs = cs_full[:, : obj.w]
                nc.tensor.matmul(cs, lhsT=sel2, rhs=obj.a, start=True, stop=True)
                csl = obj.lnbuf[:, nev, obj.lcol0 : obj.lcol0 + obj.w]
                nc.scalar.activation(out=csl, in_=cs, func=AF.Copy)
                bcx = rsc.tile([2, obj.w], bf16, tag="bcx")
                with nc.allow_low_precision(reason="rescale factor; drift lands in tracked shifts"):
                    nc.vector.reciprocal(bcx, cs)
                psb_full = pmisc.tile([128, 64], f32, tag="psb")
                psb = psb_full[:, : obj.w]
                nc.tensor.matmul(psb, lhsT=bcw, rhs=bcx, start=True, stop=True)
                an = endp.tile([128, obj.w], f32, tag=f"an{obj.name}")
                nc.vector.tensor_mul(an, psb, obj.a)
                return an

            anF0 = final_norm(F0, nev_f)
            anF1 = final_norm(F1, nev_f)
            anB0 = final_norm(B0, nev_b)

            gm = endp.tile([128, 64], f32)
            nc.vector.memset(gm, 0.0)
            nc.vector.memset(gm[0:2, :], 1.0)
            nc.vector.memset(gm[64:66, :], 1.0)
            nc.vector.tensor_copy(out=gm[:, 0:W_b], in_=anB0)

            m0 = endp.tile([128, 64], f32)
            nc.vector.tensor_mul(m0[:, 0:32], anF0, gm[:, 0:32])
            nc.vector.tensor_mul(m0[:, 32:64], anF1, gm[:, 32:64])
            psd = pmisc.tile([2, 64], f32, tag="cs")
            nc.tensor.matmul(psd, lhsT=sel2f, rhs=m0, start=True, stop=True)
            lnd = endp.tile([2, 64], f32)
            nc.scalar.activation(out=lnd, in_=psd, func=AF.Ln)

            # 2^-32 prescale keeps the Ln arg inside ScalarE's +-2^64 domain;
            # the +32ln2 per event is folded into the final bias constant
            lnbuf_f = endp.tile([2, nev_f + 1, 64], f32)
            nc.scalar.activation(
                out=lnbuf_f, in_=csbuf_f, func=AF.Ln, scale=float(2.0 ** -32)
            )
            lnbuf_b = endp.tile([2, nev_b + 1, W_b], f32)
            nc.scalar.activation(
                out=lnbuf_b, in_=csbuf_b, func=AF.Ln, scale=float(2.0 ** -32)
            )
            ssf = endp.tile([2, 64], f32)
            nc.vector.tensor_reduce(
                out=ssf, in_=lnbuf_f.rearrange("p a b -> p b a"),
                axis=AX.X, op=ALU.add,
            )
            ssb = endp.tile([2, W_b], f32)
            nc.vector.tensor_reduce(
                out=ssb, in_=lnbuf_b.rearrange("p a b -> p b a"),
                axis=AX.X, op=ALU.add,
            )
            tot = endp.tile([2, 64], f32)
            nc.vector.tensor_add(tot, lnd, ssf)
            nc.vector.tensor_add(tot[:, 0:W_b], tot[:, 0:W_b], ssb)
            red = endp.tile([2, 1], f32)
            nc.vector.tensor_reduce(out=red, in_=tot, axis=AX.X, op=ALU.add)
            ones2 = endp.tile([2, 1], f32)
            nc.vector.memset(ones2, 1.0)
            zps = pmisc.tile([1, 1], f32, tag="sc1")
            nc.tensor.matmul(zps, lhsT=red, rhs=ones2, start=True, stop=True)
            bias = (
                BC * (LN2_32 * (nev_f + 1) - 1000.0 + CBIAS * H)
                + 2.0 * W_b * (LN2_32 * (nev_b + 1) + CBIAS * NJB)
            )
            szl = endp.tile([1, 1], f32)
            nc.scalar.activation(
                out=szl, in_=zps, func=AF.Copy, bias=float(bias), scale=1.0
            )
            bcc = endp.tile([1, 1], f32)
            nc.scalar.activation(
                out=bcc, in_=bcount_s, func=AF.Copy, bias=0.0, scale=float(CBIAS)
            )
            nc.vector.tensor_sub(szl, szl, bcc)

            ge_ps = pmisc.tile([1, 1], f32, tag="sc1")
            nc.tensor.matmul(
                ge_ps, lhsT=gacc, rhs=ones_col, start=True, stop=True
            )
            gesb = endp.tile([1, 1], f32)
            nc.vector.tensor_copy(out=gesb, in_=ge_ps)
            gt_ps = pmisc.tile([1, 1], f32, tag="sc1")
            nc.tensor.matmul(
                gt_ps, lhsT=gt, rhs=ones_col[0:K, :], start=True, stop=True
            )

            fin = endp.tile([1, 8], f32)
            nc.vector.tensor_sub(fin[:, 0:1], szl, gesb)
            nc.vector.tensor_sub(fin[:, 0:1], fin[:, 0:1], gt_ps)
            nc.vector.tensor_copy(out=fin[:, 1:2], in_=szl)
            nc.vector.tensor_copy(out=fin[:, 2:3], in_=gesb)
            nc.vector.tensor_copy(out=fin[:, 3:4], in_=gt_ps)
            nc.vector.memset(fin[:, 4:8], 0.0)
            nc.sync.dma_start(out=out_d[:, :], in_=fin)

    nc.compile()
    return nc


def _get_program(W_b):
    if W_b not in _PROGRAM_CACHE:
        _PROGRAM_CACHE[W_b] = _build_program(W_b)
    return _PROGRAM_CACHE[W_b]


# --------------------------------------------------------------------------
# entry point
# --------------------------------------------------------------------------

def kernel(pred, ref, seq_len, transitions):
    from concourse.bass_utils import run_bass_kernel_spmd

    obs_f, obs_b, gold, cmat, W_b, bcount = _pack(pred, ref, seq_len)
    trans_f = np.ascontiguousarray(
        np.asarray(transitions, dtype=np.float32)[np.ix_(PERM, PERM)]
    )
    # plumbing as constant fills (mirrors the reference's own -10000 fills):
    # fwd: rows 0/1 (from end/keep-alive) -> 0.0 so exp()=1 keep-alive rows
    transf_p = trans_f.copy()
    transf_p[0, :] = 0.0
    transf_p[1, :] = 0.0
    # bwd stationary (lhsT = E^T with hold/inject plumbing):
    #   row1 = e1 (hold), [0,0]=1, [0,1]=0; rows>=2 cols 0/1 already -10000
    transb_p = np.ascontiguousarray(trans_f.T)
    transb_p[1, :] = -10000.0
    transb_p[1, 1] = 0.0
    transb_p[0, 0] = 0.0
    transb_p[0, 1] = -10000.0
    # partition-broadcast stationary pattern [2, 128]
    bcw_np = np.zeros((2, 128), np.float32)
    bcw_np[0, 0:64] = 1.0
    bcw_np[1, 64:128] = 1.0

    nc = _get_program(W_b)
    in_maps = [
        {
            "obsf": np.ascontiguousarray(obs_f[c]),
            "obsb": np.ascontiguousarray(obs_b[c]),
            "gold": np.ascontiguousarray(gold[c]),
            "trans": trans_f,
            "transfp": transf_p,
            "transb": transb_p,
            "bcwp": bcw_np,
            "cmat": np.ascontiguousarray(cmat[c]),
            "bcount": np.ascontiguousarray(bcount[c]),
        }
        for c in range(NCORES)
    ]
    total = np.float64(np.nan)
    for _attempt in range(3):
        res = run_bass_kernel_spmd(
            nc, in_maps, list(range(NCORES)),
            trace=bool(os.environ.get("BASS_TRACE")),
        )
        if res.exec_time_ns is not None:
            print(f"HW exec time: {res.exec_time_ns} ns")
        total = np.float64(0.0)
        for c in range(NCORES):
            total += np.float64(res.results[c]["out"][0, 0])
        if np.isfinite(total):
            break
    return np.array(np.float32(total))
